# revision 17
# baseline (speedup 1.0000x reference)
"""ACT (adaptive computation time) kernel for 8 TRN2 NeuronCores.

Sharding: data-parallel over batch (8 batches -> 1 per core). All tensors
kept in transposed layout [d, tokens] on device so both FFN matmuls use
natural-layout weights as lhsT and float32r (full-rate exact fp32).
Halting vectors live as rows of one [16, 1024] SBUF tile. The scalar
`active` gate is an AllReduce(max) of any(hp<0.9 & nu<6); commits are
exact g/(1-g) blends so skipped steps are bit-exact no-ops.
"""

import numpy as np

import concourse.bass as bass
import concourse.bacc as bacc
import concourse.tile as tile
import concourse.mybir as mybir
from concourse.alu_op_type import AluOpType as alu
from concourse.bass_utils import run_bass_kernel_spmd

P = 128
D = 1024
H = 4096
T = 1024          # tokens per core (= seq_len, one batch row per core)
KD = D // P       # 8
KH = H // P       # 32
TC = 512          # token chunk (matmul moving free dim / PSUM bank)
NCH = T // TC     # 2
NCORES = 8
FP = mybir.dt.float32
FPR = mybir.dt.float32r  # fp32r = 11-bit mantissa, full-rate PE (f32 is 1/4 rate)
USE_FPR = True   # matmul operand tiles carry fp32r; outputs stay f32
MMDT = FPR if USE_FPR else FP
TH = float(np.float32(1.0 - 0.1))   # 0.9f, matches jax f32 compare
AF = mybir.ActivationFunctionType

_CACHE: dict = {}
_LAST_EXEC_NS = None


def _build(max_hops: int):
    nc = bacc.Bacc(None, target_bir_lowering=False)

    state_p = nc.declare_dram_parameter("state_t", [D, T], FP, isOutput=False)
    time_p = nc.declare_dram_parameter("time_t", [D, T], FP, isOutput=False)
    pos_p = nc.declare_dram_parameter("pos_t", [D, max_hops], FP, isOutput=False)
    wp_p = nc.declare_dram_parameter("w_p", [D, 1], FP, isOutput=False)
    bp_p = nc.declare_dram_parameter("b_p", [1, 1], FP, isOutput=False)
    w1_p = nc.declare_dram_parameter("W1", [D, H], MMDT, isOutput=False)
    b1_p = nc.declare_dram_parameter("b1", [H], FP, isOutput=False)
    w2_p = nc.declare_dram_parameter("W2", [H, D], MMDT, isOutput=False)
    b2_p = nc.declare_dram_parameter("b2", [D], FP, isOutput=False)
    tw_p = nc.declare_dram_parameter("tw", [P, T // P], FP, isOutput=False)
    bpst_p = nc.declare_dram_parameter("bp_step", [1, max_hops], FP, isOutput=False)
    sel_p = nc.declare_dram_parameter("sel", [P, 4], FP, isOutput=False)
    out_prev = nc.declare_dram_parameter("out_prev", [D, T], FP, isOutput=True)
    out_rem = nc.declare_dram_parameter("out_rem", [1, T], FP, isOutput=True)
    out_nu = nc.declare_dram_parameter("out_nu", [1, T], FP, isOutput=True)

    w1_r = w1_p.rearrange("(ko p) h -> p ko h", p=P)
    w2_r = w2_p.rearrange("(ho p) d -> p ho d", p=P)
    time_r = time_p.rearrange("(ko p) t -> ko p t", p=P)

    VE = nc.vector
    SE = nc.scalar
    TE = nc.tensor

    with tile.TileContext(nc) as tc:
        with (
            tc.tile_pool(name="persist", bufs=1) as persist,
            tc.tile_pool(name="hpool", bufs=1) as hpool,
            tc.tile_pool(name="w1s", bufs=2) as w1s,
            tc.tile_pool(name="w2s", bufs=2) as w2s,
            tc.tile_pool(name="tstream", bufs=2) as tstream,
            tc.tile_pool(name="pstream", bufs=3) as pstream,
            tc.tile_pool(name="stt", bufs=2) as stt,
            tc.tile_pool(name="psum_mm", bufs=3, space="PSUM") as psum_mm,
            tc.tile_pool(name="psum_o", bufs=2, space="PSUM") as psum_o,
            tc.tile_pool(name="psum_bc", bufs=1, space="PSUM") as psum_bc,
            tc.tile_pool(name="psum_p", bufs=1, space="PSUM") as psum_p,
            tc.tile_pool(name="dram", bufs=1, space="DRAM") as drampool,
        ):
            state_sb = persist.tile([P, KD, T], FP)       # 4 MB
            s_sb = persist.tile([P, KD, T], MMDT)           # 4 MB
            h_sb = hpool.tile([P, KH, TC], MMDT)            # 8 MB
            TT = T // P  # 8 token-tiles: vec[pi, po] = v[po*P + pi]
            hp = persist.tile([P, TT], FP)
            rem = persist.tile([P, TT], FP)
            nu = persist.tile([P, TT], FP)
            pvec = persist.tile([P, TT], FP)
            sr = persist.tile([P, TT], FP)
            acc = persist.tile([P, TT], FP)
            nh = persist.tile([P, TT], FP)
            sr2 = persist.tile([P, TT], FP)
            tA = persist.tile([P, TT], FP)
            hpc = persist.tile([P, TT], FP)
            remn = persist.tile([P, TT], FP)
            tB = persist.tile([P, TT], FP)
            nuc = persist.tile([P, TT], FP)
            uw8 = persist.tile([P, TT], FP)
            i1 = persist.tile([P, TT], FP)
            i1r = persist.tile([P, 1], FP)
            s1v = persist.tile([P, TT], FP)
            uw_row = persist.tile([1, T], FP)
            z8 = persist.tile([1, 8], FP)
            ones_col = persist.tile([P, 1], FP)
            tw_sb = persist.tile([P, T // P], FP)
            bpst_sb = persist.tile([1, max_hops], FP)
            bpst_bc = persist.tile([P, max_hops], FP)
            twb = persist.tile([P, T // P], FP)
            uw_bc = persist.tile([P, T], FP)
            g_sb = persist.tile([1, 1], FP)
            g_i32 = persist.tile([1, 1], mybir.dt.int32)
            ck_i32 = persist.tile([1, 4], mybir.dt.int32)
            ckf = persist.tile([1, 4], FP)
            sel_sb = persist.tile([P, 4], FP)
            ind = persist.tile([1, 1], FP)
            ones_sb = persist.tile([1, P], FP)
            bp_sb = persist.tile([1, 1], FP)
            b1_sb = persist.tile([P, KH], FP)
            b2_sb = persist.tile([P, KD], FP)
            wp_sb = persist.tile([P, KD], FP)
            pos_sb = persist.tile([P, KD, max_hops], FP)

            prev_dram = drampool.tile([D, T], FP)
            uwd = drampool.tile([P, T // P], FP)
            cc_in = drampool.tile([1, 8], FP)
            cc_out = drampool.tile([1, 8], FP)
            prev_r = prev_dram.rearrange("(ko p) t -> ko p t", p=P)
            outprev_r = out_prev.rearrange("(ko p) t -> ko p t", p=P)

            # ---- init ----
            for t_ in (hp, rem, nu):
                VE.memset(t_[:], 0.0)
            VE.memset(ones_sb[:], 1.0)
            VE.memset(ones_col[:], 1.0)
            VE.memset(z8[:], 0.0)
            nc.sync.dma_start(cc_in[:], z8[:])
            nc.sync.dma_start(bp_sb[:], bp_p[:])
            nc.sync.dma_start(b1_sb[:], b1_p.rearrange("(ho p) -> p ho", p=P))
            nc.sync.dma_start(b2_sb[:], b2_p.rearrange("(ko p) -> p ko", p=P))
            nc.sync.dma_start(wp_sb[:], wp_p.rearrange("(ko p) one -> p (ko one)", p=P))
            nc.sync.dma_start(pos_sb[:], pos_p.rearrange("(ko p) s -> p ko s", p=P))
            nc.sync.dma_start(state_sb[:], state_p.rearrange("(ko p) t -> p ko t", p=P))
            nc.sync.dma_start(tw_sb[:], tw_p[:])
            nc.sync.dma_start(bpst_sb[:], bpst_p[:])
            nc.sync.dma_start(sel_sb[:], sel_p[:])
            pbp = psum_bc.tile([P, TC], FP, tag="pbc")
            TE.matmul(pbp[:, 0:max_hops], lhsT=ones_sb[:], rhs=bpst_sb[:],
                      start=True, stop=True)
            VE.tensor_copy(bpst_bc[:], pbp[:, 0:max_hops])
            # zero-init prev in DRAM
            for ko in range(KD):
                for c0 in range(NCH):
                    zt = pstream.tile([P, TC], FP, tag="pv")
                    VE.memset(zt[:], 0.0)
                    nc.sync.dma_start(prev_r[ko, :, c0 * TC:(c0 + 1) * TC], zt[:])

            GATE_ENGS = bass.OrderedSet([
                mybir.EngineType.PE, mybir.EngineType.Activation,
                mybir.EngineType.DVE, mybir.EngineType.SP,
            ])
            gate_regs = nc.alloc_registers("gate", GATE_ENGS)
            ckregs = [nc.alloc_registers(f"ck{i}", GATE_ENGS) for i in range(4)]

            def emit_ind():
                # indicator for the NEXT step's gate (reads committed hp/nu)
                VE.tensor_scalar(i1[:], hp[:], TH, None, alu.is_lt)
                VE.tensor_scalar(s1v[:], nu[:], float(max_hops), None, alu.is_lt)
                VE.tensor_tensor(i1[:], i1[:], s1v[:], alu.mult)
                VE.tensor_reduce(i1r[:], i1[:], mybir.AxisListType.X, alu.max)
                pind = psum_bc.tile([P, TC], FP, tag="pbc")
                TE.matmul(pind[0:1, 0:1], lhsT=ones_col[:], rhs=i1r[:],
                          start=True, stop=True)
                VE.tensor_copy(ind[:], pind[0:1, 0:1])
                nc.sync.dma_start(cc_in[0:1, 0:1], ind[:])

            def emit_body(step):
                # ---- s = state + time + pos[step] ----
                for ko in range(KD):
                    tt = tstream.tile([P, T], FP, tag="time")
                    nc.sync.dma_start(tt[:], time_r[ko])
                    VE.tensor_add(s_sb[:, ko], state_sb[:, ko], tt[:])
                    SE.activation(s_sb[:, ko], s_sb[:, ko], AF.Identity,
                                  bias=pos_sb[:, ko, step:step + 1])

                # ---- p = sigmoid(state@w_p + [time@w_p + b_p + pos@w_p]) ----
                # d-partials on DVE (uw_bc as scratch), partition-reduce via
                # one ones-matmul per chunk, repack [1,T]->[128,TT] via DRAM
                VE.tensor_scalar_add(twb[:], tw_sb[:], bpst_bc[:, step:step + 1])
                VE.tensor_scalar_mul(uw_bc[:], state_sb[:, 0], wp_sb[:, 0:1])
                for k in range(1, KD):
                    VE.scalar_tensor_tensor(uw_bc[:], state_sb[:, k],
                                            wp_sb[:, k:k + 1], uw_bc[:],
                                            alu.mult, alu.add)
                for c in range(NCH):
                    pp = psum_p.tile([1, TC], FP, tag="pp")
                    TE.matmul(pp[:], lhsT=ones_col[:],
                              rhs=uw_bc[:, c * TC:(c + 1) * TC],
                              start=True, stop=True)
                    VE.tensor_copy(uw_row[:, c * TC:(c + 1) * TC], pp[:])
                nc.sync.dma_start(
                    uwd.rearrange("(one p) po -> one (p po)", one=1), uw_row[:])
                nc.sync.dma_start(acc[:], uwd[:])
                VE.tensor_add(acc[:], acc[:], twb[:])
                SE.activation(pvec[:], acc[:], AF.Sigmoid)

                # ---- halting updates ----
                VE.tensor_scalar(sr[:], hp[:], 1.0, None, alu.is_lt)
                VE.tensor_tensor(tA[:], pvec[:], sr[:], alu.mult)
                VE.tensor_add(acc[:], hp[:], tA[:])
                VE.tensor_scalar(nh[:], acc[:], TH, None, alu.is_gt)
                VE.tensor_tensor(nh[:], nh[:], sr[:], alu.mult)
                VE.tensor_scalar(sr2[:], acc[:], TH, None, alu.is_le)
                VE.tensor_tensor(sr2[:], sr2[:], sr[:], alu.mult)
                VE.tensor_tensor(tA[:], pvec[:], sr2[:], alu.mult)
                VE.tensor_add(hpc[:], hp[:], tA[:])
                VE.tensor_scalar(tB[:], hpc[:], -1.0, 1.0, alu.mult, alu.add)
                VE.tensor_tensor(tB[:], nh[:], tB[:], alu.mult)
                VE.tensor_add(remn[:], rem[:], tB[:])
                VE.tensor_tensor(tB[:], nh[:], remn[:], alu.mult)
                VE.tensor_add(hpc[:], hpc[:], tB[:])
                VE.tensor_add(nuc[:], nu[:], sr2[:])
                VE.tensor_add(nuc[:], nuc[:], nh[:])
                VE.tensor_add(uw8[:], tA[:], tB[:])
                if step >= 2:
                    # per-256-token-chunk any(hp_old < TH); hp not yet committed
                    VE.tensor_scalar(i1[:], hp[:], TH, None, alu.is_lt)
                    VE.tensor_reduce(i1r[:], i1[:], mybir.AxisListType.X, alu.max)
                    pcs = psum_p.tile([1, TC], FP, tag="pp")
                    TE.matmul(pcs[0:1, 0:4], lhsT=i1r[:, 0:1], rhs=sel_sb[:],
                              start=True, stop=True)
                    VE.tensor_copy(ckf[:], pcs[0:1, 0:4])
                    VE.tensor_copy(ck_i32[:], ckf[:])
                    for ci in range(4):
                        nc.regs_load(ckregs[ci], ck_i32[0:1, ci:ci + 1])
                nc.sync.dma_start(uwd[:], uw8[:])
                nc.sync.dma_start(
                    uw_row[:], uwd.rearrange("(one p) po -> one (p po)", one=1))
                # direct commits (step is known-active inside the gate)
                VE.tensor_copy(hp[:], hpc[:])
                VE.tensor_copy(rem[:], remn[:])
                VE.tensor_copy(nu[:], nuc[:])
                if step < max_hops - 1:
                    emit_ind()  # early: AllReduce overlaps this step's FFN
                # broadcast uw across partitions
                for c in range(NCH):
                    puw = psum_bc.tile([P, TC], FP, tag="pbc")
                    TE.matmul(puw[:], lhsT=ones_sb[:],
                              rhs=uw_row[:, c * TC:(c + 1) * TC],
                              start=True, stop=True)
                    VE.tensor_copy(uw_bc[:, c * TC:(c + 1) * TC], puw[:])

                # ---- FFN + commits, chunked over tokens ----
                def ffn_chunk(cs, tcw):
                    for hh in range(KH):
                        w1t = w1s.tile([P, KD, P], MMDT, tag="w1")
                        nc.sync.dma_start(w1t[:], w1_r[:, :, hh * P:(hh + 1) * P])
                        ps = psum_mm.tile([P, TC], FP, tag="mm1")
                        for k in range(KD):
                            TE.matmul(ps[:, :tcw], lhsT=w1t[:, k],
                                      rhs=s_sb[:, k, cs],
                                      start=(k == 0), stop=(k == KD - 1))
                        SE.activation(h_sb[:, hh, :tcw], ps[:, :tcw], AF.Relu,
                                      bias=b1_sb[:, hh:hh + 1])
                    for dd in range(KD):
                        po = psum_o.tile([P, TC], FP, tag="mm2")
                        for half in range(2):
                            w2t = w2s.tile([P, KH // 2, P], MMDT, tag="w2")
                            nc.sync.dma_start(
                                w2t[:], w2_r[:, half * (KH // 2):(half + 1) * (KH // 2),
                                             dd * P:(dd + 1) * P])
                            for kk in range(KH // 2):
                                k = half * (KH // 2) + kk
                                TE.matmul(po[:, :tcw], lhsT=w2t[:, kk],
                                          rhs=h_sb[:, k, :tcw],
                                          start=(k == 0), stop=(k == KH - 1))
                        # state' = s_t (direct; also feeds prev blend below)
                        SE.activation(state_sb[:, dd, cs], po[:, :tcw], AF.Identity,
                                      bias=b2_sb[:, dd:dd + 1])
                        # prev' = s_t*uw + prev*(1-uw)
                        pv = pstream.tile([P, TC], FP, tag="pv")
                        nc.sync.dma_start(pv[:, :tcw], prev_r[dd, :, cs])
                        t1 = stt.tile([P, TC], FP, tag="t1")
                        VE.tensor_tensor(t1[:, :tcw], state_sb[:, dd, cs],
                                         uw_bc[:, cs], alu.mult)
                        t2 = stt.tile([P, TC], FP, tag="t2")
                        VE.tensor_tensor(t2[:, :tcw], pv[:, :tcw], uw_bc[:, cs],
                                         alu.mult)
                        VE.tensor_sub(t2[:, :tcw], pv[:, :tcw], t2[:, :tcw])
                        VE.tensor_add(t1[:, :tcw], t1[:, :tcw], t2[:, :tcw])
                        nc.sync.dma_start(prev_r[dd, :, cs], t1[:, :tcw])

                if step >= 2:
                    for ci in range(4):
                        with tc.If(nc.snap(ckregs[ci]) > 0):
                            ffn_chunk(slice(ci * 256, (ci + 1) * 256), 256)
                    return
                for c in range(NCH):
                    ffn_chunk(slice(c * TC, (c + 1) * TC), TC)

            # step 0: hp==0 everywhere -> gate is statically open
            emit_body(0)
            for step in range(1, max_hops):
                nc.gpsimd.collective_compute(
                    "AllReduce",
                    alu.add,
                    ins=[cc_in[:].opt()],
                    outs=[cc_out[:].opt()],
                    replica_groups=[list(range(NCORES))],
                )
                nc.sync.dma_start(g_sb[:], cc_out[0:1, 0:1])
                VE.tensor_copy(g_i32[:], g_sb[:])  # f32 count -> int32
                nc.regs_load(gate_regs, g_i32[0:1, 0:1])
                with tc.If(nc.snap(gate_regs) > 0):
                    emit_body(step)

            # unconditional epilogue: prev -> out_prev, rem/nu out
            for ko in range(KD):
                for c0 in range(NCH):
                    fin = pstream.tile([P, TC], FP, tag="pv")
                    nc.sync.dma_start(fin[:], prev_r[ko, :, c0 * TC:(c0 + 1) * TC])
                    nc.sync.dma_start(outprev_r[ko, :, c0 * TC:(c0 + 1) * TC], fin[:])
            nc.sync.dma_start(
                out_rem.rearrange("one (p po) -> (one p) po", p=P), rem[:])
            nc.sync.dma_start(
                out_nu.rearrange("one (p po) -> (one p) po", p=P), nu[:])

    nc.finalize()
    return nc


def prepare_in_maps(inputs):
    mh = int(np.asarray(inputs["max_hops"]))
    state = np.asarray(inputs["state"], np.float32)
    time_signal = np.asarray(inputs["time_signal"], np.float32)
    position_signal = np.asarray(inputs["position_signal"], np.float32)
    w_p = np.asarray(inputs["w_p"], np.float32)
    b_p = np.asarray(inputs["b_p"], np.float32)
    W1 = np.asarray(inputs["W1"], np.float32)
    b1 = np.asarray(inputs["b1"], np.float32)
    W2 = np.asarray(inputs["W2"], np.float32)
    b2 = np.asarray(inputs["b2"], np.float32)
    time_t = np.ascontiguousarray(time_signal[0].T)            # [D, T]
    pos_t = np.ascontiguousarray(position_signal[0].T)         # [D, mh]
    tw = (time_signal[0].astype(np.float64) @ w_p.astype(np.float64))[:, 0]
    tw = tw.astype(np.float32).reshape(P, T // P)
    bp_step = (b_p.astype(np.float64)[0]
               + position_signal[0].astype(np.float64) @ w_p.astype(np.float64)[:, 0])
    bp_step = bp_step.astype(np.float32).reshape(1, mh)
    sel = np.zeros((P, 4), np.float32)
    for ci in range(4):
        sel[ci * 32:(ci + 1) * 32, ci] = 1.0
    shared = {
        "sel": sel,
        "tw": tw,
        "bp_step": bp_step,
        "time_t": time_t,
        "pos_t": pos_t,
        "w_p": np.ascontiguousarray(w_p),
        "b_p": b_p.reshape(1, 1),
        "W1": np.ascontiguousarray(W1),
        "b1": np.ascontiguousarray(b1),
        "W2": np.ascontiguousarray(W2),
        "b2": np.ascontiguousarray(b2),
    }
    in_maps = []
    for c in range(NCORES):
        m = dict(shared)
        m["state_t"] = np.ascontiguousarray(state[c].T)
        in_maps.append(m)
    return in_maps


def unshard(results):
    prev = np.stack([results[c]["out_prev"].T for c in range(NCORES)])
    rem_o = np.stack([results[c]["out_rem"][0] for c in range(NCORES)])
    nu_o = np.stack([results[c]["out_nu"][0] for c in range(NCORES)])
    return prev, (rem_o, nu_o)


def kernel(**inputs):
    mh = int(np.asarray(inputs["max_hops"]))
    if mh not in _CACHE:
        _CACHE[mh] = _build(mh)
    nc = _CACHE[mh]
    in_maps = prepare_in_maps(inputs)

    res = run_bass_kernel_spmd(nc, in_maps, core_ids=list(range(NCORES)))
    global _LAST_EXEC_NS
    _LAST_EXEC_NS = res.exec_time_ns
    results = res.results
    return unshard(results)


# revision 19
# speedup vs baseline: 1.1046x; 1.1046x over previous
"""ACT (adaptive computation time) kernel for 8 TRN2 NeuronCores.

Sharding: data-parallel over batch (8 batches -> 1 per core). All tensors
kept in transposed layout [d, tokens] on device so both FFN matmuls use
natural-layout weights as lhsT and float32r (full-rate exact fp32).
Halting vectors live as rows of one [16, 1024] SBUF tile. The scalar
`active` gate is an AllReduce(max) of any(hp<0.9 & nu<6); commits are
exact g/(1-g) blends so skipped steps are bit-exact no-ops.
"""

import numpy as np

import concourse.bass as bass
import concourse.bacc as bacc
import concourse.tile as tile
import concourse.mybir as mybir
from concourse.alu_op_type import AluOpType as alu
from concourse.bass_utils import run_bass_kernel_spmd

P = 128
D = 1024
H = 4096
T = 1024          # tokens per core (= seq_len, one batch row per core)
KD = D // P       # 8
KH = H // P       # 32
TC = 512          # token chunk (matmul moving free dim / PSUM bank)
NCH = T // TC     # 2
NCORES = 8
FP = mybir.dt.float32
FPR = mybir.dt.float32r  # fp32r = 11-bit mantissa, full-rate PE (f32 is 1/4 rate)
USE_FPR = True   # matmul operand tiles carry fp32r; outputs stay f32
MMDT = FPR if USE_FPR else FP
TH = float(np.float32(1.0 - 0.1))   # 0.9f, matches jax f32 compare
AF = mybir.ActivationFunctionType

_CACHE: dict = {}
_LAST_EXEC_NS = None


def _build(max_hops: int):
    nc = bacc.Bacc(None, target_bir_lowering=False)

    state_p = nc.declare_dram_parameter("state_t", [D, T], FP, isOutput=False)
    time_p = nc.declare_dram_parameter("time_t", [D, T], FP, isOutput=False)
    pos_p = nc.declare_dram_parameter("pos_t", [D, max_hops], FP, isOutput=False)
    wp_p = nc.declare_dram_parameter("w_p", [D, 1], FP, isOutput=False)
    bp_p = nc.declare_dram_parameter("b_p", [1, 1], FP, isOutput=False)
    w1_p = nc.declare_dram_parameter("W1", [D, H], MMDT, isOutput=False)
    b1_p = nc.declare_dram_parameter("b1", [H], FP, isOutput=False)
    w2_p = nc.declare_dram_parameter("W2", [H, D], MMDT, isOutput=False)
    b2_p = nc.declare_dram_parameter("b2", [D], FP, isOutput=False)
    tw_p = nc.declare_dram_parameter("tw", [P, T // P], FP, isOutput=False)
    bpst_p = nc.declare_dram_parameter("bp_step", [1, max_hops], FP, isOutput=False)
    out_prev = nc.declare_dram_parameter("out_prev", [D, T], FP, isOutput=True)
    out_rem = nc.declare_dram_parameter("out_rem", [1, T], FP, isOutput=True)
    out_nu = nc.declare_dram_parameter("out_nu", [1, T], FP, isOutput=True)

    w1_r = w1_p.rearrange("(ko p) h -> p ko h", p=P)
    w2_r = w2_p.rearrange("(ho p) d -> p ho d", p=P)
    time_r = time_p.rearrange("(ko p) t -> ko p t", p=P)

    VE = nc.vector
    SE = nc.scalar
    TE = nc.tensor

    with tile.TileContext(nc) as tc:
        with (
            tc.tile_pool(name="persist", bufs=1) as persist,
            tc.tile_pool(name="hpool", bufs=1) as hpool,
            tc.tile_pool(name="w1s", bufs=2) as w1s,
            tc.tile_pool(name="w2s", bufs=2) as w2s,
            tc.tile_pool(name="tstream", bufs=2) as tstream,
            tc.tile_pool(name="pstream", bufs=3) as pstream,
            tc.tile_pool(name="stt", bufs=2) as stt,
            tc.tile_pool(name="psum_mm", bufs=3, space="PSUM") as psum_mm,
            tc.tile_pool(name="psum_o", bufs=2, space="PSUM") as psum_o,
            tc.tile_pool(name="psum_bc", bufs=1, space="PSUM") as psum_bc,
            tc.tile_pool(name="psum_p", bufs=1, space="PSUM") as psum_p,
            tc.tile_pool(name="dram", bufs=1, space="DRAM") as drampool,
        ):
            state_sb = persist.tile([P, KD, T], FP)       # 4 MB
            s_sb = persist.tile([P, KD, T], MMDT)           # 4 MB
            h_sb = hpool.tile([P, KH, TC], MMDT)            # 8 MB
            TT = T // P  # 8 token-tiles: vec[pi, po] = v[po*P + pi]
            hp = persist.tile([P, TT], FP)
            rem = persist.tile([P, TT], FP)
            nu = persist.tile([P, TT], FP)
            pvec = persist.tile([P, TT], FP)
            sr = persist.tile([P, TT], FP)
            acc = persist.tile([P, TT], FP)
            nh = persist.tile([P, TT], FP)
            sr2 = persist.tile([P, TT], FP)
            tA = persist.tile([P, TT], FP)
            hpc = persist.tile([P, TT], FP)
            remn = persist.tile([P, TT], FP)
            tB = persist.tile([P, TT], FP)
            nuc = persist.tile([P, TT], FP)
            uw8 = persist.tile([P, TT], FP)
            i1 = persist.tile([P, TT], FP)
            i1r = persist.tile([P, 1], FP)
            s1v = persist.tile([P, TT], FP)
            uw_row = persist.tile([1, T], FP)
            z8 = persist.tile([1, 8], FP)
            ones_col = persist.tile([P, 1], FP)
            tw_sb = persist.tile([P, T // P], FP)
            bpst_sb = persist.tile([1, max_hops], FP)
            bpst_bc = persist.tile([P, max_hops], FP)
            twb = persist.tile([P, T // P], FP)
            uw_bc = persist.tile([P, T], FP)
            g_sb = persist.tile([1, 1], FP)
            g_i32 = persist.tile([1, 1], mybir.dt.int32)
            ind = persist.tile([1, 1], FP)
            ones_sb = persist.tile([1, P], FP)
            bp_sb = persist.tile([1, 1], FP)
            b1_sb = persist.tile([P, KH], FP)
            b2_sb = persist.tile([P, KD], FP)
            wp_sb = persist.tile([P, KD], FP)
            pos_sb = persist.tile([P, KD, max_hops], FP)

            prev_dram = drampool.tile([D, T], FP)
            uwd = drampool.tile([P, T // P], FP)
            cc_in = drampool.tile([1, 8], FP)
            cc_out = drampool.tile([1, 8], FP)
            prev_r = prev_dram.rearrange("(ko p) t -> ko p t", p=P)
            outprev_r = out_prev.rearrange("(ko p) t -> ko p t", p=P)

            # ---- init ----
            for t_ in (hp, rem, nu):
                VE.memset(t_[:], 0.0)
            VE.memset(ones_sb[:], 1.0)
            VE.memset(ones_col[:], 1.0)
            VE.memset(z8[:], 0.0)
            nc.sync.dma_start(cc_in[:], z8[:])
            nc.sync.dma_start(bp_sb[:], bp_p[:])
            nc.sync.dma_start(b1_sb[:], b1_p.rearrange("(ho p) -> p ho", p=P))
            nc.sync.dma_start(b2_sb[:], b2_p.rearrange("(ko p) -> p ko", p=P))
            nc.sync.dma_start(wp_sb[:], wp_p.rearrange("(ko p) one -> p (ko one)", p=P))
            nc.sync.dma_start(pos_sb[:], pos_p.rearrange("(ko p) s -> p ko s", p=P))
            nc.sync.dma_start(state_sb[:], state_p.rearrange("(ko p) t -> p ko t", p=P))
            nc.sync.dma_start(tw_sb[:], tw_p[:])
            nc.sync.dma_start(bpst_sb[:], bpst_p[:])
            pbp = psum_bc.tile([P, TC], FP, tag="pbc")
            TE.matmul(pbp[:, 0:max_hops], lhsT=ones_sb[:], rhs=bpst_sb[:],
                      start=True, stop=True)
            VE.tensor_copy(bpst_bc[:], pbp[:, 0:max_hops])
            # zero-init prev in DRAM
            for ko in range(KD):
                for c0 in range(NCH):
                    zt = pstream.tile([P, TC], FP, tag="pv")
                    VE.memset(zt[:], 0.0)
                    nc.sync.dma_start(prev_r[ko, :, c0 * TC:(c0 + 1) * TC], zt[:])

            GATE_ENGS = bass.OrderedSet([
                mybir.EngineType.PE, mybir.EngineType.Activation,
                mybir.EngineType.DVE, mybir.EngineType.SP,
            ])
            gate_regs = nc.alloc_registers("gate", GATE_ENGS)

            def emit_ind():
                # indicator for the NEXT step's gate (reads committed hp/nu)
                VE.tensor_scalar(i1[:], hp[:], TH, None, alu.is_lt)
                VE.tensor_scalar(s1v[:], nu[:], float(max_hops), None, alu.is_lt)
                VE.tensor_tensor(i1[:], i1[:], s1v[:], alu.mult)
                VE.tensor_reduce(i1r[:], i1[:], mybir.AxisListType.X, alu.max)
                pind = psum_bc.tile([P, TC], FP, tag="pbc")
                TE.matmul(pind[0:1, 0:1], lhsT=ones_col[:], rhs=i1r[:],
                          start=True, stop=True)
                VE.tensor_copy(ind[:], pind[0:1, 0:1])
                nc.sync.dma_start(cc_in[0:1, 0:1], ind[:])

            def emit_body(step):
                # ---- s = state + time + pos[step] ----
                for ko in range(KD):
                    tt = tstream.tile([P, T], FP, tag="time")
                    nc.sync.dma_start(tt[:], time_r[ko])
                    VE.tensor_add(s_sb[:, ko], state_sb[:, ko], tt[:])
                    SE.activation(s_sb[:, ko], s_sb[:, ko], AF.Identity,
                                  bias=pos_sb[:, ko, step:step + 1])

                # ---- p = sigmoid(state@w_p + [time@w_p + b_p + pos@w_p]) ----
                # d-partials on DVE (uw_bc as scratch), partition-reduce via
                # one ones-matmul per chunk, repack [1,T]->[128,TT] via DRAM
                VE.tensor_scalar_add(twb[:], tw_sb[:], bpst_bc[:, step:step + 1])
                VE.tensor_scalar_mul(uw_bc[:], state_sb[:, 0], wp_sb[:, 0:1])
                for k in range(1, KD):
                    VE.scalar_tensor_tensor(uw_bc[:], state_sb[:, k],
                                            wp_sb[:, k:k + 1], uw_bc[:],
                                            alu.mult, alu.add)
                for c in range(NCH):
                    pp = psum_p.tile([1, TC], FP, tag="pp")
                    TE.matmul(pp[:], lhsT=ones_col[:],
                              rhs=uw_bc[:, c * TC:(c + 1) * TC],
                              start=True, stop=True)
                    VE.tensor_copy(uw_row[:, c * TC:(c + 1) * TC], pp[:])
                nc.sync.dma_start(
                    uwd.rearrange("(one p) po -> one (p po)", one=1), uw_row[:])
                nc.sync.dma_start(acc[:], uwd[:])
                VE.tensor_add(acc[:], acc[:], twb[:])
                SE.activation(pvec[:], acc[:], AF.Sigmoid)

                # ---- halting updates ----
                VE.tensor_scalar(sr[:], hp[:], 1.0, None, alu.is_lt)
                VE.tensor_tensor(tA[:], pvec[:], sr[:], alu.mult)
                VE.tensor_add(acc[:], hp[:], tA[:])
                VE.tensor_scalar(nh[:], acc[:], TH, None, alu.is_gt)
                VE.tensor_tensor(nh[:], nh[:], sr[:], alu.mult)
                VE.tensor_scalar(sr2[:], acc[:], TH, None, alu.is_le)
                VE.tensor_tensor(sr2[:], sr2[:], sr[:], alu.mult)
                VE.tensor_tensor(tA[:], pvec[:], sr2[:], alu.mult)
                VE.tensor_add(hpc[:], hp[:], tA[:])
                VE.tensor_scalar(tB[:], hpc[:], -1.0, 1.0, alu.mult, alu.add)
                VE.tensor_tensor(tB[:], nh[:], tB[:], alu.mult)
                VE.tensor_add(remn[:], rem[:], tB[:])
                VE.tensor_tensor(tB[:], nh[:], remn[:], alu.mult)
                VE.tensor_add(hpc[:], hpc[:], tB[:])
                VE.tensor_add(nuc[:], nu[:], sr2[:])
                VE.tensor_add(nuc[:], nuc[:], nh[:])
                VE.tensor_add(uw8[:], tA[:], tB[:])
                nc.sync.dma_start(uwd[:], uw8[:])
                nc.sync.dma_start(
                    uw_row[:], uwd.rearrange("(one p) po -> one (p po)", one=1))
                # direct commits (step is known-active inside the gate)
                VE.tensor_copy(hp[:], hpc[:])
                VE.tensor_copy(rem[:], remn[:])
                VE.tensor_copy(nu[:], nuc[:])
                if step < max_hops - 1:
                    emit_ind()  # early: AllReduce overlaps this step's FFN
                # broadcast uw across partitions
                for c in range(NCH):
                    puw = psum_bc.tile([P, TC], FP, tag="pbc")
                    TE.matmul(puw[:], lhsT=ones_sb[:],
                              rhs=uw_row[:, c * TC:(c + 1) * TC],
                              start=True, stop=True)
                    VE.tensor_copy(uw_bc[:, c * TC:(c + 1) * TC], puw[:])

                # ---- FFN + commits, chunked over tokens ----
                def ffn_chunk(cs, tcw):
                    for hh in range(KH):
                        w1t = w1s.tile([P, KD, P], MMDT, tag="w1")
                        nc.sync.dma_start(w1t[:], w1_r[:, :, hh * P:(hh + 1) * P])
                        ps = psum_mm.tile([P, TC], FP, tag="mm1")
                        for k in range(KD):
                            TE.matmul(ps[:, :tcw], lhsT=w1t[:, k],
                                      rhs=s_sb[:, k, cs],
                                      start=(k == 0), stop=(k == KD - 1))
                        SE.activation(h_sb[:, hh, :tcw], ps[:, :tcw], AF.Relu,
                                      bias=b1_sb[:, hh:hh + 1])
                    for dd in range(KD):
                        po = psum_o.tile([P, TC], FP, tag="mm2")
                        for half in range(2):
                            w2t = w2s.tile([P, KH // 2, P], MMDT, tag="w2")
                            nc.sync.dma_start(
                                w2t[:], w2_r[:, half * (KH // 2):(half + 1) * (KH // 2),
                                             dd * P:(dd + 1) * P])
                            for kk in range(KH // 2):
                                k = half * (KH // 2) + kk
                                TE.matmul(po[:, :tcw], lhsT=w2t[:, kk],
                                          rhs=h_sb[:, k, :tcw],
                                          start=(k == 0), stop=(k == KH - 1))
                        # state' = s_t (direct; also feeds prev blend below)
                        SE.activation(state_sb[:, dd, cs], po[:, :tcw], AF.Identity,
                                      bias=b2_sb[:, dd:dd + 1])
                        # prev' = s_t*uw + prev*(1-uw)
                        pv = pstream.tile([P, TC], FP, tag="pv")
                        nc.sync.dma_start(pv[:, :tcw], prev_r[dd, :, cs])
                        t1 = stt.tile([P, TC], FP, tag="t1")
                        VE.tensor_tensor(t1[:, :tcw], state_sb[:, dd, cs],
                                         uw_bc[:, cs], alu.mult)
                        t2 = stt.tile([P, TC], FP, tag="t2")
                        VE.tensor_tensor(t2[:, :tcw], pv[:, :tcw], uw_bc[:, cs],
                                         alu.mult)
                        VE.tensor_sub(t2[:, :tcw], pv[:, :tcw], t2[:, :tcw])
                        VE.tensor_add(t1[:, :tcw], t1[:, :tcw], t2[:, :tcw])
                        nc.sync.dma_start(prev_r[dd, :, cs], t1[:, :tcw])

                for c in range(NCH):
                    ffn_chunk(slice(c * TC, (c + 1) * TC), TC)

            # step 0: hp==0 everywhere -> gate is statically open
            emit_body(0)
            for step in range(1, max_hops):
                nc.gpsimd.collective_compute(
                    "AllReduce",
                    alu.add,
                    ins=[cc_in[:].opt()],
                    outs=[cc_out[:].opt()],
                    replica_groups=[list(range(NCORES))],
                )
                nc.sync.dma_start(g_sb[:], cc_out[0:1, 0:1])
                VE.tensor_copy(g_i32[:], g_sb[:])  # f32 count -> int32
                nc.regs_load(gate_regs, g_i32[0:1, 0:1])
                with tc.If(nc.snap(gate_regs) > 0):
                    emit_body(step)

            # unconditional epilogue: prev -> out_prev, rem/nu out
            for ko in range(KD):
                for c0 in range(NCH):
                    fin = pstream.tile([P, TC], FP, tag="pv")
                    nc.sync.dma_start(fin[:], prev_r[ko, :, c0 * TC:(c0 + 1) * TC])
                    nc.sync.dma_start(outprev_r[ko, :, c0 * TC:(c0 + 1) * TC], fin[:])
            nc.sync.dma_start(
                out_rem.rearrange("one (p po) -> (one p) po", p=P), rem[:])
            nc.sync.dma_start(
                out_nu.rearrange("one (p po) -> (one p) po", p=P), nu[:])

    nc.finalize()
    return nc


def prepare_in_maps(inputs):
    mh = int(np.asarray(inputs["max_hops"]))
    state = np.asarray(inputs["state"], np.float32)
    time_signal = np.asarray(inputs["time_signal"], np.float32)
    position_signal = np.asarray(inputs["position_signal"], np.float32)
    w_p = np.asarray(inputs["w_p"], np.float32)
    b_p = np.asarray(inputs["b_p"], np.float32)
    W1 = np.asarray(inputs["W1"], np.float32)
    b1 = np.asarray(inputs["b1"], np.float32)
    W2 = np.asarray(inputs["W2"], np.float32)
    b2 = np.asarray(inputs["b2"], np.float32)
    time_t = np.ascontiguousarray(time_signal[0].T)            # [D, T]
    pos_t = np.ascontiguousarray(position_signal[0].T)         # [D, mh]
    tw = (time_signal[0].astype(np.float64) @ w_p.astype(np.float64))[:, 0]
    tw = tw.astype(np.float32).reshape(P, T // P)
    bp_step = (b_p.astype(np.float64)[0]
               + position_signal[0].astype(np.float64) @ w_p.astype(np.float64)[:, 0])
    bp_step = bp_step.astype(np.float32).reshape(1, mh)
    shared = {
        "tw": tw,
        "bp_step": bp_step,
        "time_t": time_t,
        "pos_t": pos_t,
        "w_p": np.ascontiguousarray(w_p),
        "b_p": b_p.reshape(1, 1),
        "W1": np.ascontiguousarray(W1),
        "b1": np.ascontiguousarray(b1),
        "W2": np.ascontiguousarray(W2),
        "b2": np.ascontiguousarray(b2),
    }
    in_maps = []
    for c in range(NCORES):
        m = dict(shared)
        m["state_t"] = np.ascontiguousarray(state[c].T)
        in_maps.append(m)
    return in_maps


def unshard(results):
    prev = np.stack([results[c]["out_prev"].T for c in range(NCORES)])
    rem_o = np.stack([results[c]["out_rem"][0] for c in range(NCORES)])
    nu_o = np.stack([results[c]["out_nu"][0] for c in range(NCORES)])
    return prev, (rem_o, nu_o)


def kernel(**inputs):
    import os

    os.environ.pop("BASS_TRACE", None)  # profiling only via explicit trace=True
    mh = int(np.asarray(inputs["max_hops"]))
    if mh not in _CACHE:
        _CACHE[mh] = _build(mh)
    nc = _CACHE[mh]
    in_maps = prepare_in_maps(inputs)

    res = run_bass_kernel_spmd(nc, in_maps, core_ids=list(range(NCORES)))
    global _LAST_EXEC_NS
    _LAST_EXEC_NS = res.exec_time_ns
    results = res.results
    return unshard(results)


# revision 21
# speedup vs baseline: 1.2880x; 1.1660x over previous
"""ACT (adaptive computation time) kernel for 8 TRN2 NeuronCores.

Sharding: data-parallel over batch (8 batches -> 1 per core). All tensors
kept in transposed layout [d, tokens] on device so both FFN matmuls use
natural-layout weights as lhsT, with float32r operands (full PE rate;
outputs accumulate in f32 PSUM). Halting vectors use a token-on-partition
[128, 8] layout. p is computed by linearity as state@w_p (exact f32 via
DVE partials + one ones-matmul reduce) plus host-precomputed time@w_p and
b_p + pos@w_p terms, so halting decisions are fp32-exact. The scalar
`active` gate is a 32-byte AllReduce; each step after the first runs
under a tc.If on that gate, so dead trailing steps are skipped on device
(the reference's early-break emulation). prev streams through DRAM; the
last gate may be skipped, so an unconditional epilogue copies prev to
out_prev.
"""

import numpy as np

import concourse.bass as bass
import concourse.bacc as bacc
import concourse.tile as tile
import concourse.mybir as mybir
from concourse.alu_op_type import AluOpType as alu
from concourse.bass_utils import run_bass_kernel_spmd

P = 128
D = 1024
H = 4096
T = 1024          # tokens per core (= seq_len, one batch row per core)
KD = D // P       # 8
KH = H // P       # 32
TC = 512          # token chunk (matmul moving free dim / PSUM bank)
NCH = T // TC     # 2
NCORES = 8
FP = mybir.dt.float32
FPR = mybir.dt.float32r  # fp32r = 11-bit mantissa, full-rate PE (f32 is 1/4 rate)
USE_FPR = True   # matmul operand tiles carry fp32r; outputs stay f32
MMDT = FPR if USE_FPR else FP
TH = float(np.float32(1.0 - 0.1))   # 0.9f, matches jax f32 compare
AF = mybir.ActivationFunctionType

_CACHE: dict = {}
_LAST_EXEC_NS = None


def _build(max_hops: int):
    nc = bacc.Bacc(None, target_bir_lowering=False)

    state_p = nc.declare_dram_parameter("state_t", [D, T], FP, isOutput=False)
    time_p = nc.declare_dram_parameter("time_t", [D, T], FP, isOutput=False)
    pos_p = nc.declare_dram_parameter("pos_t", [D, max_hops], FP, isOutput=False)
    wp_p = nc.declare_dram_parameter("w_p", [D, 1], FP, isOutput=False)
    bp_p = nc.declare_dram_parameter("b_p", [1, 1], FP, isOutput=False)
    w1_p = nc.declare_dram_parameter("W1", [D, H], MMDT, isOutput=False)
    b1_p = nc.declare_dram_parameter("b1", [H], FP, isOutput=False)
    w2_p = nc.declare_dram_parameter("W2", [H, D], MMDT, isOutput=False)
    b2_p = nc.declare_dram_parameter("b2", [D], FP, isOutput=False)
    tw_p = nc.declare_dram_parameter("tw", [P, T // P], FP, isOutput=False)
    bpst_p = nc.declare_dram_parameter("bp_step", [1, max_hops], FP, isOutput=False)
    out_prev = nc.declare_dram_parameter("out_prev", [D, T], FP, isOutput=True)
    out_rem = nc.declare_dram_parameter("out_rem", [1, T], FP, isOutput=True)
    out_nu = nc.declare_dram_parameter("out_nu", [1, T], FP, isOutput=True)

    w1_r = w1_p.rearrange("(ko p) h -> p ko h", p=P)
    w2_r = w2_p.rearrange("(ho p) d -> p ho d", p=P)
    time_r = time_p.rearrange("(ko p) t -> ko p t", p=P)

    VE = nc.vector
    SE = nc.scalar
    TE = nc.tensor

    with tile.TileContext(nc) as tc:
        with (
            tc.tile_pool(name="persist", bufs=1) as persist,
            tc.tile_pool(name="hpool", bufs=1) as hpool,
            tc.tile_pool(name="w1s", bufs=3) as w1s,
            tc.tile_pool(name="w2s", bufs=2) as w2s,
            tc.tile_pool(name="tstream", bufs=2) as tstream,
            tc.tile_pool(name="pstream", bufs=3) as pstream,
            tc.tile_pool(name="stt", bufs=2) as stt,
            tc.tile_pool(name="psum_mm", bufs=4, space="PSUM") as psum_mm,
            tc.tile_pool(name="psum_o", bufs=2, space="PSUM") as psum_o,
            tc.tile_pool(name="psum_bc", bufs=1, space="PSUM") as psum_bc,
            tc.tile_pool(name="psum_p", bufs=1, space="PSUM") as psum_p,
            tc.tile_pool(name="dram", bufs=1, space="DRAM") as drampool,
        ):
            state_sb = persist.tile([P, KD, T], FP)       # 4 MB
            s_sb = persist.tile([P, KD, T], MMDT)           # 4 MB
            h_sb = hpool.tile([P, KH, TC], MMDT)            # 8 MB
            TT = T // P  # 8 token-tiles: vec[pi, po] = v[po*P + pi]
            hp = persist.tile([P, TT], FP)
            rem = persist.tile([P, TT], FP)
            nu = persist.tile([P, TT], FP)
            pvec = persist.tile([P, TT], FP)
            sr = persist.tile([P, TT], FP)
            acc = persist.tile([P, TT], FP)
            nh = persist.tile([P, TT], FP)
            sr2 = persist.tile([P, TT], FP)
            tA = persist.tile([P, TT], FP)
            hpc = persist.tile([P, TT], FP)
            remn = persist.tile([P, TT], FP)
            tB = persist.tile([P, TT], FP)
            nuc = persist.tile([P, TT], FP)
            uw8 = persist.tile([P, TT], FP)
            i1 = persist.tile([P, TT], FP)
            i1r = persist.tile([P, 1], FP)
            s1v = persist.tile([P, TT], FP)
            uw_row = persist.tile([1, T], FP)
            z8 = persist.tile([1, 8], FP)
            ones_col = persist.tile([P, 1], FP)
            tw_sb = persist.tile([P, T // P], FP)
            bpst_sb = persist.tile([1, max_hops], FP)
            bpst_bc = persist.tile([P, max_hops], FP)
            twb = persist.tile([P, T // P], FP)
            uw_bc = persist.tile([P, T], FP)
            g_sb = persist.tile([1, 1], FP)
            g_i32 = persist.tile([1, 1], mybir.dt.int32)
            ind = persist.tile([1, 1], FP)
            ones_sb = persist.tile([1, P], FP)
            bp_sb = persist.tile([1, 1], FP)
            b1_sb = persist.tile([P, KH], FP)
            b2_sb = persist.tile([P, KD], FP)
            wp_sb = persist.tile([P, KD], FP)
            pos_sb = persist.tile([P, KD, max_hops], FP)

            prev_dram = drampool.tile([D, T], FP)
            uwd = drampool.tile([P, T // P], FP)
            cc_in = drampool.tile([1, 8], FP)
            cc_out = drampool.tile([1, 8], FP)
            prev_r = prev_dram.rearrange("(ko p) t -> ko p t", p=P)
            outprev_r = out_prev.rearrange("(ko p) t -> ko p t", p=P)

            # ---- init ----
            for t_ in (hp, rem, nu):
                VE.memset(t_[:], 0.0)
            VE.memset(ones_sb[:], 1.0)
            VE.memset(ones_col[:], 1.0)
            VE.memset(z8[:], 0.0)
            nc.sync.dma_start(cc_in[:], z8[:])
            nc.sync.dma_start(bp_sb[:], bp_p[:])
            nc.sync.dma_start(b1_sb[:], b1_p.rearrange("(ho p) -> p ho", p=P))
            nc.sync.dma_start(b2_sb[:], b2_p.rearrange("(ko p) -> p ko", p=P))
            nc.sync.dma_start(wp_sb[:], wp_p.rearrange("(ko p) one -> p (ko one)", p=P))
            nc.sync.dma_start(pos_sb[:], pos_p.rearrange("(ko p) s -> p ko s", p=P))
            nc.sync.dma_start(state_sb[:], state_p.rearrange("(ko p) t -> p ko t", p=P))
            nc.sync.dma_start(tw_sb[:], tw_p[:])
            nc.sync.dma_start(bpst_sb[:], bpst_p[:])
            pbp = psum_bc.tile([P, TC], FP, tag="pbc")
            TE.matmul(pbp[:, 0:max_hops], lhsT=ones_sb[:], rhs=bpst_sb[:],
                      start=True, stop=True)
            VE.tensor_copy(bpst_bc[:], pbp[:, 0:max_hops])
            # zero-init prev in DRAM
            for ko in range(KD):
                for c0 in range(NCH):
                    zt = pstream.tile([P, TC], FP, tag="pv")
                    VE.memset(zt[:], 0.0)
                    nc.sync.dma_start(prev_r[ko, :, c0 * TC:(c0 + 1) * TC], zt[:])

            GATE_ENGS = bass.OrderedSet([
                mybir.EngineType.PE, mybir.EngineType.Activation,
                mybir.EngineType.DVE, mybir.EngineType.SP,
            ])
            gate_regs = nc.alloc_registers("gate", GATE_ENGS)

            def emit_ind():
                # indicator for the NEXT step's gate (reads committed hp/nu)
                VE.tensor_scalar(i1[:], hp[:], TH, None, alu.is_lt)
                VE.tensor_scalar(s1v[:], nu[:], float(max_hops), None, alu.is_lt)
                VE.tensor_tensor(i1[:], i1[:], s1v[:], alu.mult)
                VE.tensor_reduce(i1r[:], i1[:], mybir.AxisListType.X, alu.max)
                pind = psum_bc.tile([P, TC], FP, tag="pbc")
                TE.matmul(pind[0:1, 0:1], lhsT=ones_col[:], rhs=i1r[:],
                          start=True, stop=True)
                VE.tensor_copy(ind[:], pind[0:1, 0:1])
                nc.sync.dma_start(cc_in[0:1, 0:1], ind[:])

            def emit_body(step):
                # ---- s = state + time + pos[step] ----
                for ko in range(KD):
                    tt = tstream.tile([P, T], FP, tag="time")
                    nc.sync.dma_start(tt[:], time_r[ko])
                    VE.tensor_add(s_sb[:, ko], state_sb[:, ko], tt[:])
                    SE.activation(s_sb[:, ko], s_sb[:, ko], AF.Identity,
                                  bias=pos_sb[:, ko, step:step + 1])

                # ---- p = sigmoid(state@w_p + [time@w_p + b_p + pos@w_p]) ----
                # d-partials on DVE (uw_bc as scratch), partition-reduce via
                # one ones-matmul per chunk, repack [1,T]->[128,TT] via DRAM
                VE.tensor_scalar_add(twb[:], tw_sb[:], bpst_bc[:, step:step + 1])
                VE.tensor_scalar_mul(uw_bc[:], state_sb[:, 0], wp_sb[:, 0:1])
                for k in range(1, KD):
                    VE.scalar_tensor_tensor(uw_bc[:], state_sb[:, k],
                                            wp_sb[:, k:k + 1], uw_bc[:],
                                            alu.mult, alu.add)
                for c in range(NCH):
                    pp = psum_p.tile([1, TC], FP, tag="pp")
                    TE.matmul(pp[:], lhsT=ones_col[:],
                              rhs=uw_bc[:, c * TC:(c + 1) * TC],
                              start=True, stop=True)
                    VE.tensor_copy(uw_row[:, c * TC:(c + 1) * TC], pp[:])
                nc.sync.dma_start(
                    uwd.rearrange("(one p) po -> one (p po)", one=1), uw_row[:])
                nc.sync.dma_start(acc[:], uwd[:])
                VE.tensor_add(acc[:], acc[:], twb[:])
                SE.activation(pvec[:], acc[:], AF.Sigmoid)

                # ---- halting updates ----
                VE.tensor_scalar(sr[:], hp[:], 1.0, None, alu.is_lt)
                VE.tensor_tensor(tA[:], pvec[:], sr[:], alu.mult)
                VE.tensor_add(acc[:], hp[:], tA[:])
                VE.tensor_scalar(nh[:], acc[:], TH, None, alu.is_gt)
                VE.tensor_tensor(nh[:], nh[:], sr[:], alu.mult)
                VE.tensor_scalar(sr2[:], acc[:], TH, None, alu.is_le)
                VE.tensor_tensor(sr2[:], sr2[:], sr[:], alu.mult)
                VE.tensor_tensor(tA[:], pvec[:], sr2[:], alu.mult)
                VE.tensor_add(hpc[:], hp[:], tA[:])
                VE.tensor_scalar(tB[:], hpc[:], -1.0, 1.0, alu.mult, alu.add)
                VE.tensor_tensor(tB[:], nh[:], tB[:], alu.mult)
                VE.tensor_add(remn[:], rem[:], tB[:])
                VE.tensor_tensor(tB[:], nh[:], remn[:], alu.mult)
                VE.tensor_add(hpc[:], hpc[:], tB[:])
                VE.tensor_add(nuc[:], nu[:], sr2[:])
                VE.tensor_add(nuc[:], nuc[:], nh[:])
                VE.tensor_add(uw8[:], tA[:], tB[:])
                nc.sync.dma_start(uwd[:], uw8[:])
                nc.sync.dma_start(
                    uw_row[:], uwd.rearrange("(one p) po -> one (p po)", one=1))
                # direct commits (step is known-active inside the gate)
                VE.tensor_copy(hp[:], hpc[:])
                VE.tensor_copy(rem[:], remn[:])
                VE.tensor_copy(nu[:], nuc[:])
                if step < max_hops - 1:
                    emit_ind()  # early: AllReduce overlaps this step's FFN
                # broadcast uw across partitions
                for c in range(NCH):
                    puw = psum_bc.tile([P, TC], FP, tag="pbc")
                    TE.matmul(puw[:], lhsT=ones_sb[:],
                              rhs=uw_row[:, c * TC:(c + 1) * TC],
                              start=True, stop=True)
                    VE.tensor_copy(uw_bc[:, c * TC:(c + 1) * TC], puw[:])

                # ---- FFN + commits, chunked over tokens ----
                def ffn_chunk(cs, tcw):
                    for hh in range(KH):
                        w1t = w1s.tile([P, KD, P], MMDT, tag="w1")
                        nc.sync.dma_start(w1t[:], w1_r[:, :, hh * P:(hh + 1) * P])
                        ps = psum_mm.tile([P, TC], FP, tag="mm1")
                        for k in range(KD):
                            TE.matmul(ps[:, :tcw], lhsT=w1t[:, k],
                                      rhs=s_sb[:, k, cs],
                                      start=(k == 0), stop=(k == KD - 1))
                        SE.activation(h_sb[:, hh, :tcw], ps[:, :tcw], AF.Relu,
                                      bias=b1_sb[:, hh:hh + 1])
                    for dd in range(KD):
                        po = psum_o.tile([P, TC], FP, tag="mm2")
                        for half in range(2):
                            w2t = w2s.tile([P, KH // 2, P], MMDT, tag="w2")
                            nc.sync.dma_start(
                                w2t[:], w2_r[:, half * (KH // 2):(half + 1) * (KH // 2),
                                             dd * P:(dd + 1) * P])
                            for kk in range(KH // 2):
                                k = half * (KH // 2) + kk
                                TE.matmul(po[:, :tcw], lhsT=w2t[:, kk],
                                          rhs=h_sb[:, k, :tcw],
                                          start=(k == 0), stop=(k == KH - 1))
                        # state' = s_t (direct; also feeds prev blend below)
                        SE.activation(state_sb[:, dd, cs], po[:, :tcw], AF.Identity,
                                      bias=b2_sb[:, dd:dd + 1])
                        # prev' = s_t*uw + prev*(1-uw)
                        pv = pstream.tile([P, TC], FP, tag="pv")
                        nc.sync.dma_start(pv[:, :tcw], prev_r[dd, :, cs])
                        t1 = stt.tile([P, TC], FP, tag="t1")
                        VE.tensor_tensor(t1[:, :tcw], state_sb[:, dd, cs],
                                         uw_bc[:, cs], alu.mult)
                        t2 = stt.tile([P, TC], FP, tag="t2")
                        VE.tensor_tensor(t2[:, :tcw], pv[:, :tcw], uw_bc[:, cs],
                                         alu.mult)
                        VE.tensor_sub(t2[:, :tcw], pv[:, :tcw], t2[:, :tcw])
                        VE.tensor_add(t1[:, :tcw], t1[:, :tcw], t2[:, :tcw])
                        nc.sync.dma_start(prev_r[dd, :, cs], t1[:, :tcw])

                for c in range(NCH):
                    ffn_chunk(slice(c * TC, (c + 1) * TC), TC)

            # step 0: hp==0 everywhere -> gate is statically open
            emit_body(0)
            for step in range(1, max_hops):
                nc.gpsimd.collective_compute(
                    "AllReduce",
                    alu.add,
                    ins=[cc_in[:].opt()],
                    outs=[cc_out[:].opt()],
                    replica_groups=[list(range(NCORES))],
                )
                nc.sync.dma_start(g_sb[:], cc_out[0:1, 0:1])
                VE.tensor_copy(g_i32[:], g_sb[:])  # f32 count -> int32
                nc.regs_load(gate_regs, g_i32[0:1, 0:1])
                with tc.If(nc.snap(gate_regs) > 0):
                    emit_body(step)

            # unconditional epilogue: prev -> out_prev, rem/nu out
            for ko in range(KD):
                for c0 in range(NCH):
                    fin = pstream.tile([P, TC], FP, tag="pv")
                    nc.sync.dma_start(fin[:], prev_r[ko, :, c0 * TC:(c0 + 1) * TC])
                    nc.sync.dma_start(outprev_r[ko, :, c0 * TC:(c0 + 1) * TC], fin[:])
            nc.sync.dma_start(
                out_rem.rearrange("one (p po) -> (one p) po", p=P), rem[:])
            nc.sync.dma_start(
                out_nu.rearrange("one (p po) -> (one p) po", p=P), nu[:])

    nc.finalize()
    return nc


def prepare_in_maps(inputs):
    mh = int(np.asarray(inputs["max_hops"]))
    state = np.asarray(inputs["state"], np.float32)
    time_signal = np.asarray(inputs["time_signal"], np.float32)
    position_signal = np.asarray(inputs["position_signal"], np.float32)
    w_p = np.asarray(inputs["w_p"], np.float32)
    b_p = np.asarray(inputs["b_p"], np.float32)
    W1 = np.asarray(inputs["W1"], np.float32)
    b1 = np.asarray(inputs["b1"], np.float32)
    W2 = np.asarray(inputs["W2"], np.float32)
    b2 = np.asarray(inputs["b2"], np.float32)
    time_t = np.ascontiguousarray(time_signal[0].T)            # [D, T]
    pos_t = np.ascontiguousarray(position_signal[0].T)         # [D, mh]
    tw = (time_signal[0].astype(np.float64) @ w_p.astype(np.float64))[:, 0]
    tw = tw.astype(np.float32).reshape(P, T // P)
    bp_step = (b_p.astype(np.float64)[0]
               + position_signal[0].astype(np.float64) @ w_p.astype(np.float64)[:, 0])
    bp_step = bp_step.astype(np.float32).reshape(1, mh)
    shared = {
        "tw": tw,
        "bp_step": bp_step,
        "time_t": time_t,
        "pos_t": pos_t,
        "w_p": np.ascontiguousarray(w_p),
        "b_p": b_p.reshape(1, 1),
        "W1": np.ascontiguousarray(W1),
        "b1": np.ascontiguousarray(b1),
        "W2": np.ascontiguousarray(W2),
        "b2": np.ascontiguousarray(b2),
    }
    in_maps = []
    for c in range(NCORES):
        m = dict(shared)
        m["state_t"] = np.ascontiguousarray(state[c].T)
        in_maps.append(m)
    return in_maps


def unshard(results):
    prev = np.stack([results[c]["out_prev"].T for c in range(NCORES)])
    rem_o = np.stack([results[c]["out_rem"][0] for c in range(NCORES)])
    nu_o = np.stack([results[c]["out_nu"][0] for c in range(NCORES)])
    return prev, (rem_o, nu_o)


def kernel(**inputs):
    import os

    os.environ.pop("BASS_TRACE", None)  # profiling only via explicit trace=True
    mh = int(np.asarray(inputs["max_hops"]))
    if mh not in _CACHE:
        _CACHE[mh] = _build(mh)
    nc = _CACHE[mh]
    in_maps = prepare_in_maps(inputs)

    res = run_bass_kernel_spmd(nc, in_maps, core_ids=list(range(NCORES)))
    global _LAST_EXEC_NS
    _LAST_EXEC_NS = res.exec_time_ns
    results = res.results
    return unshard(results)


# revision 22
# speedup vs baseline: 1.3640x; 1.0590x over previous
"""ACT (adaptive computation time) kernel for 8 TRN2 NeuronCores.

Sharding: data-parallel over batch (8 batches -> 1 per core). All tensors
kept in transposed layout [d, tokens] on device so both FFN matmuls use
natural-layout weights as lhsT, with float32r operands (full PE rate;
outputs accumulate in f32 PSUM). Halting vectors use a token-on-partition
[128, 8] layout. p is computed by linearity as state@w_p (exact f32 via
DVE partials + one ones-matmul reduce) plus host-precomputed time@w_p and
b_p + pos@w_p terms, so halting decisions are fp32-exact. The scalar
`active` gate is a 32-byte AllReduce; each step after the first runs
under a tc.If on that gate, so dead trailing steps are skipped on device
(the reference's early-break emulation). prev streams through DRAM; the
last gate may be skipped, so an unconditional epilogue copies prev to
out_prev.
"""

import numpy as np

import concourse.bass as bass
import concourse.bacc as bacc
import concourse.tile as tile
import concourse.mybir as mybir
from concourse.alu_op_type import AluOpType as alu
from concourse.bass_utils import run_bass_kernel_spmd

P = 128
D = 1024
H = 4096
T = 1024          # tokens per core (= seq_len, one batch row per core)
KD = D // P       # 8
KH = H // P       # 32
TC = 512          # token chunk (matmul moving free dim / PSUM bank)
NCH = T // TC     # 2
NCORES = 8
FP = mybir.dt.float32
FPR = mybir.dt.float32r  # fp32r = 11-bit mantissa, full-rate PE (f32 is 1/4 rate)
USE_FPR = True   # matmul operand tiles carry fp32r; outputs stay f32
MMDT = FPR if USE_FPR else FP
TH = float(np.float32(1.0 - 0.1))   # 0.9f, matches jax f32 compare
AF = mybir.ActivationFunctionType

_CACHE: dict = {}
_LAST_EXEC_NS = None


def _build(max_hops: int):
    nc = bacc.Bacc(None, target_bir_lowering=False)

    state_p = nc.declare_dram_parameter("state_t", [D, T], FP, isOutput=False)
    time_p = nc.declare_dram_parameter("time_t", [D, T], FP, isOutput=False)
    pos_p = nc.declare_dram_parameter("pos_t", [D, max_hops], FP, isOutput=False)
    wp_p = nc.declare_dram_parameter("w_p", [D, 1], FP, isOutput=False)
    bp_p = nc.declare_dram_parameter("b_p", [1, 1], FP, isOutput=False)
    w1_p = nc.declare_dram_parameter("W1", [D, H], MMDT, isOutput=False)
    b1_p = nc.declare_dram_parameter("b1", [H], FP, isOutput=False)
    w2_p = nc.declare_dram_parameter("W2", [H, D], MMDT, isOutput=False)
    b2_p = nc.declare_dram_parameter("b2", [D], FP, isOutput=False)
    tw_p = nc.declare_dram_parameter("tw", [P, T // P], FP, isOutput=False)
    bpst_p = nc.declare_dram_parameter("bp_step", [1, max_hops], FP, isOutput=False)
    out_prev = nc.declare_dram_parameter("out_prev", [D, T], FP, isOutput=True)
    out_rem = nc.declare_dram_parameter("out_rem", [1, T], FP, isOutput=True)
    out_nu = nc.declare_dram_parameter("out_nu", [1, T], FP, isOutput=True)

    w1_r = w1_p.rearrange("(ko p) h -> p ko h", p=P)
    w2_r = w2_p.rearrange("(ho p) d -> p ho d", p=P)
    time_r = time_p.rearrange("(ko p) t -> ko p t", p=P)

    VE = nc.vector
    SE = nc.scalar
    TE = nc.tensor

    with tile.TileContext(nc) as tc:
        with (
            tc.tile_pool(name="persist", bufs=1) as persist,
            tc.tile_pool(name="hpool", bufs=1) as hpool,
            tc.tile_pool(name="w1s", bufs=3) as w1s,
            tc.tile_pool(name="w2s", bufs=3) as w2s,
            tc.tile_pool(name="tstream", bufs=2) as tstream,
            tc.tile_pool(name="pstream", bufs=3) as pstream,
            tc.tile_pool(name="stt", bufs=2) as stt,
            tc.tile_pool(name="psum_mm", bufs=4, space="PSUM") as psum_mm,
            tc.tile_pool(name="psum_o", bufs=3, space="PSUM") as psum_o,
            tc.tile_pool(name="psum_bc", bufs=1, space="PSUM") as psum_bc,
            tc.tile_pool(name="dram", bufs=1, space="DRAM") as drampool,
        ):
            state_sb = persist.tile([P, KD, T], FP)       # 4 MB
            s_sb = persist.tile([P, KD, T], MMDT)           # 4 MB
            h_sb = hpool.tile([P, KH, TC], MMDT)            # 8 MB
            TT = T // P  # 8 token-tiles: vec[pi, po] = v[po*P + pi]
            hp = persist.tile([P, TT], FP)
            rem = persist.tile([P, TT], FP)
            nu = persist.tile([P, TT], FP)
            pvec = persist.tile([P, TT], FP)
            sr = persist.tile([P, TT], FP)
            acc = persist.tile([P, TT], FP)
            nh = persist.tile([P, TT], FP)
            sr2 = persist.tile([P, TT], FP)
            tA = persist.tile([P, TT], FP)
            hpc = persist.tile([P, TT], FP)
            remn = persist.tile([P, TT], FP)
            tB = persist.tile([P, TT], FP)
            nuc = persist.tile([P, TT], FP)
            uw8 = persist.tile([P, TT], FP)
            i1 = persist.tile([P, TT], FP)
            i1r = persist.tile([P, 1], FP)
            s1v = persist.tile([P, TT], FP)
            uw_row = persist.tile([1, T], FP)
            z8 = persist.tile([1, 8], FP)
            ones_col = persist.tile([P, 1], FP)
            tw_sb = persist.tile([P, T // P], FP)
            bpst_sb = persist.tile([1, max_hops], FP)
            bpst_bc = persist.tile([P, max_hops], FP)
            twb = persist.tile([P, T // P], FP)
            uw_bc = persist.tile([P, T], FP)
            g_sb = persist.tile([1, 1], FP)
            g_i32 = persist.tile([1, 1], mybir.dt.int32)
            ind = persist.tile([1, 1], FP)
            ones_sb = persist.tile([1, P], FP)
            bp_sb = persist.tile([1, 1], FP)
            b1_sb = persist.tile([P, KH], FP)
            b2_sb = persist.tile([P, KD], FP)
            wp_sb = persist.tile([P, KD], FP)
            pos_sb = persist.tile([P, KD, max_hops], FP)

            prev_dram = drampool.tile([D, T], FP)
            uwd = drampool.tile([P, T // P], FP)
            cc_in = drampool.tile([1, 8], FP)
            cc_out = drampool.tile([1, 8], FP)
            prev_r = prev_dram.rearrange("(ko p) t -> ko p t", p=P)
            outprev_r = out_prev.rearrange("(ko p) t -> ko p t", p=P)

            # ---- init ----
            for t_ in (hp, rem, nu):
                VE.memset(t_[:], 0.0)
            VE.memset(ones_sb[:], 1.0)
            VE.memset(ones_col[:], 1.0)
            VE.memset(z8[:], 0.0)
            nc.sync.dma_start(cc_in[:], z8[:])
            nc.sync.dma_start(bp_sb[:], bp_p[:])
            nc.sync.dma_start(b1_sb[:], b1_p.rearrange("(ho p) -> p ho", p=P))
            nc.sync.dma_start(b2_sb[:], b2_p.rearrange("(ko p) -> p ko", p=P))
            nc.sync.dma_start(wp_sb[:], wp_p.rearrange("(ko p) one -> p (ko one)", p=P))
            nc.sync.dma_start(pos_sb[:], pos_p.rearrange("(ko p) s -> p ko s", p=P))
            nc.sync.dma_start(state_sb[:], state_p.rearrange("(ko p) t -> p ko t", p=P))
            nc.sync.dma_start(tw_sb[:], tw_p[:])
            nc.sync.dma_start(bpst_sb[:], bpst_p[:])
            pbp = psum_bc.tile([P, TC], FP, tag="pbc")
            TE.matmul(pbp[:, 0:max_hops], lhsT=ones_sb[:], rhs=bpst_sb[:],
                      start=True, stop=True)
            VE.tensor_copy(bpst_bc[:], pbp[:, 0:max_hops])
            # zero-init prev in DRAM
            for ko in range(KD):
                for c0 in range(NCH):
                    zt = pstream.tile([P, TC], FP, tag="pv")
                    VE.memset(zt[:], 0.0)
                    nc.sync.dma_start(prev_r[ko, :, c0 * TC:(c0 + 1) * TC], zt[:])

            GATE_ENGS = bass.OrderedSet([
                mybir.EngineType.PE, mybir.EngineType.Activation,
                mybir.EngineType.DVE, mybir.EngineType.SP,
            ])
            gate_regs = nc.alloc_registers("gate", GATE_ENGS)

            def emit_ind():
                # indicator for the NEXT step's gate (reads committed hp/nu)
                VE.tensor_scalar(i1[:], hp[:], TH, None, alu.is_lt)
                VE.tensor_scalar(s1v[:], nu[:], float(max_hops), None, alu.is_lt)
                VE.tensor_tensor(i1[:], i1[:], s1v[:], alu.mult)
                VE.tensor_reduce(i1r[:], i1[:], mybir.AxisListType.X, alu.max)
                pind = psum_bc.tile([P, TC], FP, tag="pbc")
                TE.matmul(pind[0:1, 0:1], lhsT=ones_col[:], rhs=i1r[:],
                          start=True, stop=True)
                VE.tensor_copy(ind[:], pind[0:1, 0:1])
                nc.sync.dma_start(cc_in[0:1, 0:1], ind[:])

            def emit_body(step):
                # ---- s = state + time + pos[step] ----
                for ko in range(KD):
                    tt = tstream.tile([P, T], FP, tag="time")
                    nc.sync.dma_start(tt[:], time_r[ko])
                    VE.tensor_add(s_sb[:, ko], state_sb[:, ko], tt[:])
                    SE.activation(s_sb[:, ko], s_sb[:, ko], AF.Identity,
                                  bias=pos_sb[:, ko, step:step + 1])

                # ---- p = sigmoid(state@w_p + [time@w_p + b_p + pos@w_p]) ----
                # d-partials on DVE (uw_bc as scratch), partition-reduce via
                # one ones-matmul per chunk, repack [1,T]->[128,TT] via DRAM
                VE.tensor_scalar_add(twb[:], tw_sb[:], bpst_bc[:, step:step + 1])
                VE.tensor_scalar_mul(uw_bc[:], state_sb[:, 0], wp_sb[:, 0:1])
                for k in range(1, KD):
                    VE.scalar_tensor_tensor(uw_bc[:], state_sb[:, k],
                                            wp_sb[:, k:k + 1], uw_bc[:],
                                            alu.mult, alu.add)
                for c in range(NCH):
                    pp = psum_bc.tile([P, TC], FP, tag="pbc")
                    TE.matmul(pp[0:1, :], lhsT=ones_col[:],
                              rhs=uw_bc[:, c * TC:(c + 1) * TC],
                              start=True, stop=True)
                    VE.tensor_copy(uw_row[:, c * TC:(c + 1) * TC], pp[0:1, :])
                nc.sync.dma_start(
                    uwd.rearrange("(one p) po -> one (p po)", one=1), uw_row[:])
                nc.sync.dma_start(acc[:], uwd[:])
                VE.tensor_add(acc[:], acc[:], twb[:])
                SE.activation(pvec[:], acc[:], AF.Sigmoid)

                # ---- halting updates ----
                VE.tensor_scalar(sr[:], hp[:], 1.0, None, alu.is_lt)
                VE.tensor_tensor(tA[:], pvec[:], sr[:], alu.mult)
                VE.tensor_add(acc[:], hp[:], tA[:])
                VE.tensor_scalar(nh[:], acc[:], TH, None, alu.is_gt)
                VE.tensor_tensor(nh[:], nh[:], sr[:], alu.mult)
                VE.tensor_scalar(sr2[:], acc[:], TH, None, alu.is_le)
                VE.tensor_tensor(sr2[:], sr2[:], sr[:], alu.mult)
                VE.tensor_tensor(tA[:], pvec[:], sr2[:], alu.mult)
                VE.tensor_add(hpc[:], hp[:], tA[:])
                VE.tensor_scalar(tB[:], hpc[:], -1.0, 1.0, alu.mult, alu.add)
                VE.tensor_tensor(tB[:], nh[:], tB[:], alu.mult)
                VE.tensor_add(remn[:], rem[:], tB[:])
                VE.tensor_tensor(tB[:], nh[:], remn[:], alu.mult)
                VE.tensor_add(hpc[:], hpc[:], tB[:])
                VE.tensor_add(nuc[:], nu[:], sr2[:])
                VE.tensor_add(nuc[:], nuc[:], nh[:])
                VE.tensor_add(uw8[:], tA[:], tB[:])
                nc.sync.dma_start(uwd[:], uw8[:])
                nc.sync.dma_start(
                    uw_row[:], uwd.rearrange("(one p) po -> one (p po)", one=1))
                # direct commits (step is known-active inside the gate)
                VE.tensor_copy(hp[:], hpc[:])
                VE.tensor_copy(rem[:], remn[:])
                VE.tensor_copy(nu[:], nuc[:])
                if step < max_hops - 1:
                    emit_ind()  # early: AllReduce overlaps this step's FFN
                # broadcast uw across partitions
                for c in range(NCH):
                    puw = psum_bc.tile([P, TC], FP, tag="pbc")
                    TE.matmul(puw[:], lhsT=ones_sb[:],
                              rhs=uw_row[:, c * TC:(c + 1) * TC],
                              start=True, stop=True)
                    VE.tensor_copy(uw_bc[:, c * TC:(c + 1) * TC], puw[:])

                # ---- FFN + commits, chunked over tokens ----
                def ffn_chunk(cs, tcw):
                    for hh in range(KH):
                        w1t = w1s.tile([P, KD, P], MMDT, tag="w1")
                        nc.sync.dma_start(w1t[:], w1_r[:, :, hh * P:(hh + 1) * P])
                        ps = psum_mm.tile([P, TC], FP, tag="mm1")
                        for k in range(KD):
                            TE.matmul(ps[:, :tcw], lhsT=w1t[:, k],
                                      rhs=s_sb[:, k, cs],
                                      start=(k == 0), stop=(k == KD - 1))
                        SE.activation(h_sb[:, hh, :tcw], ps[:, :tcw], AF.Relu,
                                      bias=b1_sb[:, hh:hh + 1])
                    for dd in range(KD):
                        po = psum_o.tile([P, TC], FP, tag="mm2")
                        for half in range(2):
                            w2t = w2s.tile([P, KH // 2, P], MMDT, tag="w2")
                            nc.sync.dma_start(
                                w2t[:], w2_r[:, half * (KH // 2):(half + 1) * (KH // 2),
                                             dd * P:(dd + 1) * P])
                            for kk in range(KH // 2):
                                k = half * (KH // 2) + kk
                                TE.matmul(po[:, :tcw], lhsT=w2t[:, kk],
                                          rhs=h_sb[:, k, :tcw],
                                          start=(k == 0), stop=(k == KH - 1))
                        # state' = s_t (direct; also feeds prev blend below)
                        SE.activation(state_sb[:, dd, cs], po[:, :tcw], AF.Identity,
                                      bias=b2_sb[:, dd:dd + 1])
                        # prev' = s_t*uw + prev*(1-uw)
                        pv = pstream.tile([P, TC], FP, tag="pv")
                        nc.sync.dma_start(pv[:, :tcw], prev_r[dd, :, cs])
                        t1 = stt.tile([P, TC], FP, tag="t1")
                        VE.tensor_tensor(t1[:, :tcw], state_sb[:, dd, cs],
                                         uw_bc[:, cs], alu.mult)
                        t2 = stt.tile([P, TC], FP, tag="t2")
                        VE.tensor_tensor(t2[:, :tcw], pv[:, :tcw], uw_bc[:, cs],
                                         alu.mult)
                        VE.tensor_sub(t2[:, :tcw], pv[:, :tcw], t2[:, :tcw])
                        VE.tensor_add(t1[:, :tcw], t1[:, :tcw], t2[:, :tcw])
                        nc.sync.dma_start(prev_r[dd, :, cs], t1[:, :tcw])

                for c in range(NCH):
                    ffn_chunk(slice(c * TC, (c + 1) * TC), TC)

            # step 0: hp==0 everywhere -> gate is statically open
            emit_body(0)
            for step in range(1, max_hops):
                nc.gpsimd.collective_compute(
                    "AllReduce",
                    alu.add,
                    ins=[cc_in[:].opt()],
                    outs=[cc_out[:].opt()],
                    replica_groups=[list(range(NCORES))],
                )
                nc.sync.dma_start(g_sb[:], cc_out[0:1, 0:1])
                VE.tensor_copy(g_i32[:], g_sb[:])  # f32 count -> int32
                nc.regs_load(gate_regs, g_i32[0:1, 0:1])
                with tc.If(nc.snap(gate_regs) > 0):
                    emit_body(step)

            # unconditional epilogue: prev -> out_prev, rem/nu out
            for ko in range(KD):
                for c0 in range(NCH):
                    fin = pstream.tile([P, TC], FP, tag="pv")
                    nc.sync.dma_start(fin[:], prev_r[ko, :, c0 * TC:(c0 + 1) * TC])
                    nc.sync.dma_start(outprev_r[ko, :, c0 * TC:(c0 + 1) * TC], fin[:])
            nc.sync.dma_start(
                out_rem.rearrange("one (p po) -> (one p) po", p=P), rem[:])
            nc.sync.dma_start(
                out_nu.rearrange("one (p po) -> (one p) po", p=P), nu[:])

    nc.finalize()
    return nc


def prepare_in_maps(inputs):
    mh = int(np.asarray(inputs["max_hops"]))
    state = np.asarray(inputs["state"], np.float32)
    time_signal = np.asarray(inputs["time_signal"], np.float32)
    position_signal = np.asarray(inputs["position_signal"], np.float32)
    w_p = np.asarray(inputs["w_p"], np.float32)
    b_p = np.asarray(inputs["b_p"], np.float32)
    W1 = np.asarray(inputs["W1"], np.float32)
    b1 = np.asarray(inputs["b1"], np.float32)
    W2 = np.asarray(inputs["W2"], np.float32)
    b2 = np.asarray(inputs["b2"], np.float32)
    time_t = np.ascontiguousarray(time_signal[0].T)            # [D, T]
    pos_t = np.ascontiguousarray(position_signal[0].T)         # [D, mh]
    tw = (time_signal[0].astype(np.float64) @ w_p.astype(np.float64))[:, 0]
    tw = tw.astype(np.float32).reshape(P, T // P)
    bp_step = (b_p.astype(np.float64)[0]
               + position_signal[0].astype(np.float64) @ w_p.astype(np.float64)[:, 0])
    bp_step = bp_step.astype(np.float32).reshape(1, mh)
    shared = {
        "tw": tw,
        "bp_step": bp_step,
        "time_t": time_t,
        "pos_t": pos_t,
        "w_p": np.ascontiguousarray(w_p),
        "b_p": b_p.reshape(1, 1),
        "W1": np.ascontiguousarray(W1),
        "b1": np.ascontiguousarray(b1),
        "W2": np.ascontiguousarray(W2),
        "b2": np.ascontiguousarray(b2),
    }
    in_maps = []
    for c in range(NCORES):
        m = dict(shared)
        m["state_t"] = np.ascontiguousarray(state[c].T)
        in_maps.append(m)
    return in_maps


def unshard(results):
    prev = np.stack([results[c]["out_prev"].T for c in range(NCORES)])
    rem_o = np.stack([results[c]["out_rem"][0] for c in range(NCORES)])
    nu_o = np.stack([results[c]["out_nu"][0] for c in range(NCORES)])
    return prev, (rem_o, nu_o)


def kernel(**inputs):
    import os

    os.environ.pop("BASS_TRACE", None)  # profiling only via explicit trace=True
    mh = int(np.asarray(inputs["max_hops"]))
    if mh not in _CACHE:
        _CACHE[mh] = _build(mh)
    nc = _CACHE[mh]
    in_maps = prepare_in_maps(inputs)

    res = run_bass_kernel_spmd(nc, in_maps, core_ids=list(range(NCORES)))
    global _LAST_EXEC_NS
    _LAST_EXEC_NS = res.exec_time_ns
    results = res.results
    return unshard(results)


# revision 23
# speedup vs baseline: 1.3876x; 1.0173x over previous
"""ACT (adaptive computation time) kernel for 8 TRN2 NeuronCores.

Sharding: data-parallel over batch (8 batches -> 1 per core). All tensors
kept in transposed layout [d, tokens] on device so both FFN matmuls use
natural-layout weights as lhsT, with float32r operands (full PE rate;
outputs accumulate in f32 PSUM). Halting vectors use a token-on-partition
[128, 8] layout. p is computed by linearity as state@w_p (exact f32 via
DVE partials + one ones-matmul reduce) plus host-precomputed time@w_p and
b_p + pos@w_p terms, so halting decisions are fp32-exact. The scalar
`active` gate is a 32-byte AllReduce; each step after the first runs
under a tc.If on that gate, so dead trailing steps are skipped on device
(the reference's early-break emulation). prev streams through DRAM; the
last gate may be skipped, so an unconditional epilogue copies prev to
out_prev.
"""

import numpy as np

import concourse.bass as bass
import concourse.bacc as bacc
import concourse.tile as tile
import concourse.mybir as mybir
from concourse.alu_op_type import AluOpType as alu
from concourse.bass_utils import run_bass_kernel_spmd

P = 128
D = 1024
H = 4096
T = 1024          # tokens per core (= seq_len, one batch row per core)
KD = D // P       # 8
KH = H // P       # 32
TC = 512          # token chunk (matmul moving free dim / PSUM bank)
NCH = T // TC     # 2
NCORES = 8
FP = mybir.dt.float32
FPR = mybir.dt.float32r  # fp32r = 11-bit mantissa, full-rate PE (f32 is 1/4 rate)
USE_FPR = True   # matmul operand tiles carry fp32r; outputs stay f32
MMDT = FPR if USE_FPR else FP
TH = float(np.float32(1.0 - 0.1))   # 0.9f, matches jax f32 compare
AF = mybir.ActivationFunctionType

_CACHE: dict = {}
_LAST_EXEC_NS = None


def _build(max_hops: int):
    nc = bacc.Bacc(None, target_bir_lowering=False)

    state_p = nc.declare_dram_parameter("state_t", [D, T], FP, isOutput=False)
    time_p = nc.declare_dram_parameter("time_t", [D, T], FP, isOutput=False)
    pos_p = nc.declare_dram_parameter("pos_t", [D, max_hops], FP, isOutput=False)
    wp_p = nc.declare_dram_parameter("w_p", [D, 1], FP, isOutput=False)
    bp_p = nc.declare_dram_parameter("b_p", [1, 1], FP, isOutput=False)
    w1_p = nc.declare_dram_parameter("W1", [D, H], MMDT, isOutput=False)
    b1_p = nc.declare_dram_parameter("b1", [H], FP, isOutput=False)
    w2_p = nc.declare_dram_parameter("W2", [H, D], MMDT, isOutput=False)
    b2_p = nc.declare_dram_parameter("b2", [D], FP, isOutput=False)
    tw_p = nc.declare_dram_parameter("tw", [P, T // P], FP, isOutput=False)
    bpst_p = nc.declare_dram_parameter("bp_step", [1, max_hops], FP, isOutput=False)
    out_prev = nc.declare_dram_parameter("out_prev", [D, T], FP, isOutput=True)
    out_rem = nc.declare_dram_parameter("out_rem", [1, T], FP, isOutput=True)
    out_nu = nc.declare_dram_parameter("out_nu", [1, T], FP, isOutput=True)

    w1_r = w1_p.rearrange("(ko p) h -> p ko h", p=P)
    w2_r = w2_p.rearrange("(ho p) d -> p ho d", p=P)
    time_r = time_p.rearrange("(ko p) t -> ko p t", p=P)

    VE = nc.vector
    SE = nc.scalar
    TE = nc.tensor

    with tile.TileContext(nc) as tc:
        with (
            tc.tile_pool(name="persist", bufs=1) as persist,
            tc.tile_pool(name="hpool", bufs=1) as hpool,
            tc.tile_pool(name="w1s", bufs=3) as w1s,
            tc.tile_pool(name="w2s", bufs=3) as w2s,
            tc.tile_pool(name="tstream", bufs=3) as tstream,
            tc.tile_pool(name="pstream", bufs=4) as pstream,
            tc.tile_pool(name="stt", bufs=3) as stt,
            tc.tile_pool(name="psum_mm", bufs=4, space="PSUM") as psum_mm,
            tc.tile_pool(name="psum_o", bufs=3, space="PSUM") as psum_o,
            tc.tile_pool(name="psum_bc", bufs=1, space="PSUM") as psum_bc,
            tc.tile_pool(name="dram", bufs=1, space="DRAM") as drampool,
        ):
            state_sb = persist.tile([P, KD, T], FP)       # 4 MB
            s_sb = persist.tile([P, KD, T], MMDT)           # 4 MB
            h_sb = hpool.tile([P, KH, TC], MMDT)            # 8 MB
            TT = T // P  # 8 token-tiles: vec[pi, po] = v[po*P + pi]
            hp = persist.tile([P, TT], FP)
            rem = persist.tile([P, TT], FP)
            nu = persist.tile([P, TT], FP)
            pvec = persist.tile([P, TT], FP)
            sr = persist.tile([P, TT], FP)
            acc = persist.tile([P, TT], FP)
            nh = persist.tile([P, TT], FP)
            sr2 = persist.tile([P, TT], FP)
            tA = persist.tile([P, TT], FP)
            hpc = persist.tile([P, TT], FP)
            remn = persist.tile([P, TT], FP)
            tB = persist.tile([P, TT], FP)
            nuc = persist.tile([P, TT], FP)
            uw8 = persist.tile([P, TT], FP)
            i1 = persist.tile([P, TT], FP)
            i1r = persist.tile([P, 1], FP)
            s1v = persist.tile([P, TT], FP)
            uw_row = persist.tile([1, T], FP)
            z8 = persist.tile([1, 8], FP)
            ones_col = persist.tile([P, 1], FP)
            tw_sb = persist.tile([P, T // P], FP)
            bpst_sb = persist.tile([1, max_hops], FP)
            bpst_bc = persist.tile([P, max_hops], FP)
            twb = persist.tile([P, T // P], FP)
            uw_bc = persist.tile([P, T], FP)
            g_sb = persist.tile([1, 1], FP)
            g_i32 = persist.tile([1, 1], mybir.dt.int32)
            ind = persist.tile([1, 1], FP)
            ones_sb = persist.tile([1, P], FP)
            bp_sb = persist.tile([1, 1], FP)
            b1_sb = persist.tile([P, KH], FP)
            b2_sb = persist.tile([P, KD], FP)
            wp_sb = persist.tile([P, KD], FP)
            pos_sb = persist.tile([P, KD, max_hops], FP)

            prev_dram = drampool.tile([D, T], FP)
            uwd = drampool.tile([P, T // P], FP)
            cc_in = drampool.tile([1, 8], FP)
            cc_out = drampool.tile([1, 8], FP)
            prev_r = prev_dram.rearrange("(ko p) t -> ko p t", p=P)
            outprev_r = out_prev.rearrange("(ko p) t -> ko p t", p=P)

            # ---- init ----
            for t_ in (hp, rem, nu):
                VE.memset(t_[:], 0.0)
            VE.memset(ones_sb[:], 1.0)
            VE.memset(ones_col[:], 1.0)
            VE.memset(z8[:], 0.0)
            nc.sync.dma_start(cc_in[:], z8[:])
            nc.sync.dma_start(bp_sb[:], bp_p[:])
            nc.sync.dma_start(b1_sb[:], b1_p.rearrange("(ho p) -> p ho", p=P))
            nc.sync.dma_start(b2_sb[:], b2_p.rearrange("(ko p) -> p ko", p=P))
            nc.sync.dma_start(wp_sb[:], wp_p.rearrange("(ko p) one -> p (ko one)", p=P))
            nc.sync.dma_start(pos_sb[:], pos_p.rearrange("(ko p) s -> p ko s", p=P))
            nc.sync.dma_start(state_sb[:], state_p.rearrange("(ko p) t -> p ko t", p=P))
            nc.sync.dma_start(tw_sb[:], tw_p[:])
            nc.sync.dma_start(bpst_sb[:], bpst_p[:])
            pbp = psum_bc.tile([P, TC], FP, tag="pbc")
            TE.matmul(pbp[:, 0:max_hops], lhsT=ones_sb[:], rhs=bpst_sb[:],
                      start=True, stop=True)
            VE.tensor_copy(bpst_bc[:], pbp[:, 0:max_hops])
            # zero-init prev in DRAM
            for ko in range(KD):
                for c0 in range(NCH):
                    zt = pstream.tile([P, TC], FP, tag="pv")
                    VE.memset(zt[:], 0.0)
                    nc.sync.dma_start(prev_r[ko, :, c0 * TC:(c0 + 1) * TC], zt[:])

            GATE_ENGS = bass.OrderedSet([
                mybir.EngineType.PE, mybir.EngineType.Activation,
                mybir.EngineType.DVE, mybir.EngineType.SP,
            ])
            gate_regs = nc.alloc_registers("gate", GATE_ENGS)

            def emit_ind():
                # indicator for the NEXT step's gate (reads committed hp/nu)
                VE.tensor_scalar(i1[:], hp[:], TH, None, alu.is_lt)
                VE.tensor_scalar(s1v[:], nu[:], float(max_hops), None, alu.is_lt)
                VE.tensor_tensor(i1[:], i1[:], s1v[:], alu.mult)
                VE.tensor_reduce(i1r[:], i1[:], mybir.AxisListType.X, alu.max)
                pind = psum_bc.tile([P, TC], FP, tag="pbc")
                TE.matmul(pind[0:1, 0:1], lhsT=ones_col[:], rhs=i1r[:],
                          start=True, stop=True)
                VE.tensor_copy(ind[:], pind[0:1, 0:1])
                nc.sync.dma_start(cc_in[0:1, 0:1], ind[:])

            def emit_body(step):
                # ---- s = state + time + pos[step] ----
                for ko in range(KD):
                    tt = tstream.tile([P, T], FP, tag="time")
                    nc.sync.dma_start(tt[:], time_r[ko])
                    VE.tensor_add(s_sb[:, ko], state_sb[:, ko], tt[:])
                    SE.activation(s_sb[:, ko], s_sb[:, ko], AF.Identity,
                                  bias=pos_sb[:, ko, step:step + 1])

                # ---- p = sigmoid(state@w_p + [time@w_p + b_p + pos@w_p]) ----
                # d-partials on DVE (uw_bc as scratch), partition-reduce via
                # one ones-matmul per chunk, repack [1,T]->[128,TT] via DRAM
                VE.tensor_scalar_add(twb[:], tw_sb[:], bpst_bc[:, step:step + 1])
                VE.tensor_scalar_mul(uw_bc[:], state_sb[:, 0], wp_sb[:, 0:1])
                for k in range(1, KD):
                    VE.scalar_tensor_tensor(uw_bc[:], state_sb[:, k],
                                            wp_sb[:, k:k + 1], uw_bc[:],
                                            alu.mult, alu.add)
                for c in range(NCH):
                    pp = psum_bc.tile([P, TC], FP, tag="pbc")
                    TE.matmul(pp[0:1, :], lhsT=ones_col[:],
                              rhs=uw_bc[:, c * TC:(c + 1) * TC],
                              start=True, stop=True)
                    VE.tensor_copy(uw_row[:, c * TC:(c + 1) * TC], pp[0:1, :])
                nc.sync.dma_start(
                    uwd.rearrange("(one p) po -> one (p po)", one=1), uw_row[:])
                nc.sync.dma_start(acc[:], uwd[:])
                VE.tensor_add(acc[:], acc[:], twb[:])
                SE.activation(pvec[:], acc[:], AF.Sigmoid)

                # ---- halting updates ----
                VE.tensor_scalar(sr[:], hp[:], 1.0, None, alu.is_lt)
                VE.tensor_tensor(tA[:], pvec[:], sr[:], alu.mult)
                VE.tensor_add(acc[:], hp[:], tA[:])
                VE.tensor_scalar(nh[:], acc[:], TH, None, alu.is_gt)
                VE.tensor_tensor(nh[:], nh[:], sr[:], alu.mult)
                VE.tensor_scalar(sr2[:], acc[:], TH, None, alu.is_le)
                VE.tensor_tensor(sr2[:], sr2[:], sr[:], alu.mult)
                VE.tensor_tensor(tA[:], pvec[:], sr2[:], alu.mult)
                VE.tensor_add(hpc[:], hp[:], tA[:])
                VE.tensor_scalar(tB[:], hpc[:], -1.0, 1.0, alu.mult, alu.add)
                VE.tensor_tensor(tB[:], nh[:], tB[:], alu.mult)
                VE.tensor_add(remn[:], rem[:], tB[:])
                VE.tensor_tensor(tB[:], nh[:], remn[:], alu.mult)
                VE.tensor_add(hpc[:], hpc[:], tB[:])
                VE.tensor_add(nuc[:], nu[:], sr2[:])
                VE.tensor_add(nuc[:], nuc[:], nh[:])
                VE.tensor_add(uw8[:], tA[:], tB[:])
                nc.sync.dma_start(uwd[:], uw8[:])
                nc.sync.dma_start(
                    uw_row[:], uwd.rearrange("(one p) po -> one (p po)", one=1))
                # direct commits (step is known-active inside the gate)
                VE.tensor_copy(hp[:], hpc[:])
                VE.tensor_copy(rem[:], remn[:])
                VE.tensor_copy(nu[:], nuc[:])
                if step < max_hops - 1:
                    emit_ind()  # early: AllReduce overlaps this step's FFN
                # broadcast uw across partitions
                for c in range(NCH):
                    puw = psum_bc.tile([P, TC], FP, tag="pbc")
                    TE.matmul(puw[:], lhsT=ones_sb[:],
                              rhs=uw_row[:, c * TC:(c + 1) * TC],
                              start=True, stop=True)
                    VE.tensor_copy(uw_bc[:, c * TC:(c + 1) * TC], puw[:])

                # ---- FFN + commits, chunked over tokens ----
                def ffn_chunk(cs, tcw):
                    for hh in range(KH):
                        w1t = w1s.tile([P, KD, P], MMDT, tag="w1")
                        nc.sync.dma_start(w1t[:], w1_r[:, :, hh * P:(hh + 1) * P])
                        ps = psum_mm.tile([P, TC], FP, tag="mm1")
                        for k in range(KD):
                            TE.matmul(ps[:, :tcw], lhsT=w1t[:, k],
                                      rhs=s_sb[:, k, cs],
                                      start=(k == 0), stop=(k == KD - 1))
                        SE.activation(h_sb[:, hh, :tcw], ps[:, :tcw], AF.Relu,
                                      bias=b1_sb[:, hh:hh + 1])
                    for dd in range(KD):
                        po = psum_o.tile([P, TC], FP, tag="mm2")
                        for half in range(2):
                            w2t = w2s.tile([P, KH // 2, P], MMDT, tag="w2")
                            nc.sync.dma_start(
                                w2t[:], w2_r[:, half * (KH // 2):(half + 1) * (KH // 2),
                                             dd * P:(dd + 1) * P])
                            for kk in range(KH // 2):
                                k = half * (KH // 2) + kk
                                TE.matmul(po[:, :tcw], lhsT=w2t[:, kk],
                                          rhs=h_sb[:, k, :tcw],
                                          start=(k == 0), stop=(k == KH - 1))
                        # state' = s_t (direct; also feeds prev blend below)
                        SE.activation(state_sb[:, dd, cs], po[:, :tcw], AF.Identity,
                                      bias=b2_sb[:, dd:dd + 1])
                        # prev' = s_t*uw + prev*(1-uw)
                        pv = pstream.tile([P, TC], FP, tag="pv")
                        nc.sync.dma_start(pv[:, :tcw], prev_r[dd, :, cs])
                        t1 = stt.tile([P, TC], FP, tag="t1")
                        VE.tensor_tensor(t1[:, :tcw], state_sb[:, dd, cs],
                                         uw_bc[:, cs], alu.mult)
                        t2 = stt.tile([P, TC], FP, tag="t2")
                        VE.tensor_tensor(t2[:, :tcw], pv[:, :tcw], uw_bc[:, cs],
                                         alu.mult)
                        VE.tensor_sub(t2[:, :tcw], pv[:, :tcw], t2[:, :tcw])
                        VE.tensor_add(t1[:, :tcw], t1[:, :tcw], t2[:, :tcw])
                        nc.sync.dma_start(prev_r[dd, :, cs], t1[:, :tcw])

                for c in range(NCH):
                    ffn_chunk(slice(c * TC, (c + 1) * TC), TC)

            # step 0: hp==0 everywhere -> gate is statically open
            emit_body(0)
            for step in range(1, max_hops):
                nc.gpsimd.collective_compute(
                    "AllReduce",
                    alu.add,
                    ins=[cc_in[:].opt()],
                    outs=[cc_out[:].opt()],
                    replica_groups=[list(range(NCORES))],
                )
                nc.sync.dma_start(g_sb[:], cc_out[0:1, 0:1])
                VE.tensor_copy(g_i32[:], g_sb[:])  # f32 count -> int32
                nc.regs_load(gate_regs, g_i32[0:1, 0:1])
                with tc.If(nc.snap(gate_regs) > 0):
                    emit_body(step)

            # unconditional epilogue: prev -> out_prev, rem/nu out
            for ko in range(KD):
                for c0 in range(NCH):
                    fin = pstream.tile([P, TC], FP, tag="pv")
                    nc.sync.dma_start(fin[:], prev_r[ko, :, c0 * TC:(c0 + 1) * TC])
                    nc.sync.dma_start(outprev_r[ko, :, c0 * TC:(c0 + 1) * TC], fin[:])
            nc.sync.dma_start(
                out_rem.rearrange("one (p po) -> (one p) po", p=P), rem[:])
            nc.sync.dma_start(
                out_nu.rearrange("one (p po) -> (one p) po", p=P), nu[:])

    nc.finalize()
    return nc


def prepare_in_maps(inputs):
    mh = int(np.asarray(inputs["max_hops"]))
    state = np.asarray(inputs["state"], np.float32)
    time_signal = np.asarray(inputs["time_signal"], np.float32)
    position_signal = np.asarray(inputs["position_signal"], np.float32)
    w_p = np.asarray(inputs["w_p"], np.float32)
    b_p = np.asarray(inputs["b_p"], np.float32)
    W1 = np.asarray(inputs["W1"], np.float32)
    b1 = np.asarray(inputs["b1"], np.float32)
    W2 = np.asarray(inputs["W2"], np.float32)
    b2 = np.asarray(inputs["b2"], np.float32)
    time_t = np.ascontiguousarray(time_signal[0].T)            # [D, T]
    pos_t = np.ascontiguousarray(position_signal[0].T)         # [D, mh]
    tw = (time_signal[0].astype(np.float64) @ w_p.astype(np.float64))[:, 0]
    tw = tw.astype(np.float32).reshape(P, T // P)
    bp_step = (b_p.astype(np.float64)[0]
               + position_signal[0].astype(np.float64) @ w_p.astype(np.float64)[:, 0])
    bp_step = bp_step.astype(np.float32).reshape(1, mh)
    shared = {
        "tw": tw,
        "bp_step": bp_step,
        "time_t": time_t,
        "pos_t": pos_t,
        "w_p": np.ascontiguousarray(w_p),
        "b_p": b_p.reshape(1, 1),
        "W1": np.ascontiguousarray(W1),
        "b1": np.ascontiguousarray(b1),
        "W2": np.ascontiguousarray(W2),
        "b2": np.ascontiguousarray(b2),
    }
    in_maps = []
    for c in range(NCORES):
        m = dict(shared)
        m["state_t"] = np.ascontiguousarray(state[c].T)
        in_maps.append(m)
    return in_maps


def unshard(results):
    prev = np.stack([results[c]["out_prev"].T for c in range(NCORES)])
    rem_o = np.stack([results[c]["out_rem"][0] for c in range(NCORES)])
    nu_o = np.stack([results[c]["out_nu"][0] for c in range(NCORES)])
    return prev, (rem_o, nu_o)


def kernel(**inputs):
    import os

    os.environ.pop("BASS_TRACE", None)  # profiling only via explicit trace=True
    mh = int(np.asarray(inputs["max_hops"]))
    if mh not in _CACHE:
        _CACHE[mh] = _build(mh)
    nc = _CACHE[mh]
    in_maps = prepare_in_maps(inputs)

    res = run_bass_kernel_spmd(nc, in_maps, core_ids=list(range(NCORES)))
    global _LAST_EXEC_NS
    _LAST_EXEC_NS = res.exec_time_ns
    results = res.results
    return unshard(results)
